# revision 4
# baseline (speedup 1.0000x reference)
"""Multi-level ROI Align (FPN pooler, 4 levels summed) on 8 Trainium2 cores.

Strategy: shard ROIs across cores (core k: batch k//4, 128 ROIs). All gather
indices and bilinear weights are computed on host from `boxes`; the device
kernel does the heavy lifting: HBM pixel gathers (dma_gather) + weighted
scatter-reduction into 7x7 bins via PSUM-accumulating matmuls.

Per ROI, per level:
  out[bin, c] = sum_j W[j, bin] * G[j, c]
where G rows are gathered pixel vectors (C=256) and W is sparse (built on
device as fixed_pattern * per-partition scalar for L0/L1, host-baked dense
for the region-gathered L2/L3).

L0 uses 3-pixel elements addressed at even-pixel granularity (idx = flat//2)
to fit the int16 index range (200*200 = 40000 > 32767).
"""
import sys
import numpy as np

sys.path.insert(0, '/opt/trn_rl_repo')

POOLED = 7
SAMP = 2
NBIN = 49
C = 256
IMG = 800.0

# per level: H, W, scale, mode
#   mode 'tri': 3-px elems, idx=flat//2, NJ j's with 3 weight slots
#   mode 'px' : 1-px elems, corner gathers
#   mode 'reg': 1-px elems, bounding-region pixels, host-baked lhsT
L0 = dict(H=200, W=200, scale=0.25, mode='tri', NJ=512, REAL=392, NCH=4)
L1 = dict(H=100, W=100, scale=0.125, mode='px', NJ=896, REAL=784, NCH=7)
L2 = dict(H=50, W=50, scale=0.0625, mode='reg', NJ=384, REAL=324, NCH=3, WREG=18)
L3 = dict(H=25, W=25, scale=0.03125, mode='reg', NJ=128, REAL=100, NCH=1, WREG=10)
LEVELS = [L0, L1, L2, L3]

NROI_CORE = 128     # ROIs per core
NGRP = 64           # groups of 2 ROIs
GRP = 2

# padded flat pixel counts of the feature buffers
F0_ROWS = 40004     # covers 3-px elem overrun
F1_ROWS = 10000
F2_ROWS = 3400      # covers region overrun (y,x up to 66)
F3_ROWS = 900       # covers region overrun (y,x up to 33)

# const fp32 column layout (per partition)
PAT0_OFF = 0                       # [4, 49]
PAT1_OFF = PAT0_OFF + 4 * NBIN     # [7, 49]
WCOL0_OFF = PAT1_OFF + 7 * NBIN    # [128 roi * 12]
WCOL1_OFF = WCOL0_OFF + NROI_CORE * 12   # [128 roi * 7]
ID_OFF = WCOL1_OFF + NROI_CORE * 7       # [49]
CST_COLS = ID_OFF + NBIN

# idx int16 column layout (per partition), per 2-ROI group
IC0, IC1, IC2, IC3 = 64, 112, 48, 16     # cols per group per level
IDX0_OFF = 0
IDX1_OFF = IDX0_OFF + NGRP * IC0
IDX2_OFF = IDX1_OFF + NGRP * IC1
IDX3_OFF = IDX2_OFF + NGRP * IC2
IDX_COLS = IDX3_OFF + NGRP * IC3

_MODULE_CACHE = {}
TRACE = False
LAST_RESULT = {}


def _sample_meta(boxes_b, H, W, scale):
    """Per-ROI sample geometry in fp32, matching reference op order.
    boxes_b: [N, 4] fp32. Returns dict of [N,7,2] arrays."""
    f = np.float32
    b = boxes_b.astype(np.float32)
    x1 = b[:, 0] * f(scale)
    y1 = b[:, 1] * f(scale)
    x2 = b[:, 2] * f(scale)
    y2 = b[:, 3] * f(scale)
    rw = np.maximum(x2 - x1, f(1.0))
    rh = np.maximum(y2 - y1, f(1.0))
    bw = rw / f(POOLED)
    bh = rh / f(POOLED)
    g = (np.arange(POOLED, dtype=np.float32)[:, None]
         + (np.arange(SAMP, dtype=np.float32)[None, :] + f(0.5)) / f(SAMP))
    y = y1[:, None, None] + g[None] * bh[:, None, None]   # [N,7,2]
    x = x1[:, None, None] + g[None] * bw[:, None, None]
    masky = ((y >= f(-1.0)) & (y <= f(H))).astype(np.float32)
    maskx = ((x >= f(-1.0)) & (x <= f(W))).astype(np.float32)
    yc = np.clip(y, f(0.0), f(H - 1))
    xc = np.clip(x, f(0.0), f(W - 1))
    yl = np.floor(yc).astype(np.int64)
    xl = np.floor(xc).astype(np.int64)
    yh = np.minimum(yl + 1, H - 1)
    xh = np.minimum(xl + 1, W - 1)
    ly = (yc - yl.astype(np.float32)).astype(np.float32)
    lx = (xc - xl.astype(np.float32)).astype(np.float32)
    hy = (f(1.0) - ly).astype(np.float32)
    hx = (f(1.0) - lx).astype(np.float32)
    return dict(yl=yl, yh=yh, xl=xl, xh=xh, ly=ly, lx=lx, hy=hy, hx=hx,
                masky=masky, maskx=maskx, x=x, y=y)


def _build_tri(meta, lv):
    """L0: j = (row_sel, py, sy, px, sx) -> 392 3-px elems, 3 slot weights.
    Returns idx [N, NJ] int64, w [N, NJ, 3] fp32."""
    N = meta['yl'].shape[0]
    W = lv['W']
    NJ, REAL = lv['NJ'], lv['REAL']
    rows = np.stack([meta['yl'], meta['yh']], axis=1)          # [N,2,7,2] (rs)
    wys = np.stack([meta['hy'], meta['ly']], axis=1)           # [N,2,7,2]
    m = (meta['masky'][:, :, :, None, None] * meta['maskx'][:, None, None, :, :])  # [N,7,2,7,2]
    # broadcast to [N, rs, py, sy, px, sx]
    row = np.broadcast_to(rows[:, :, :, :, None, None], (N, 2, 7, 2, 7, 2))
    wy = np.broadcast_to(wys[:, :, :, :, None, None], (N, 2, 7, 2, 7, 2)).astype(np.float32)
    xl = np.broadcast_to(meta['xl'][:, None, None, None, :, :], (N, 2, 7, 2, 7, 2))
    hx = np.broadcast_to(meta['hx'][:, None, None, None, :, :], (N, 2, 7, 2, 7, 2)).astype(np.float32)
    lx = np.broadcast_to(meta['lx'][:, None, None, None, :, :], (N, 2, 7, 2, 7, 2)).astype(np.float32)
    mm = np.broadcast_to(m[:, None], (N, 2, 7, 2, 7, 2)).astype(np.float32)
    flat = row * W + xl
    idx = (flat >> 1).reshape(N, REAL)
    r = (flat & 1).astype(np.float32).reshape(N, REAL)
    wl = (wy * hx * mm * np.float32(0.25)).reshape(N, REAL)
    wh = (wy * lx * mm * np.float32(0.25)).reshape(N, REAL)
    w = np.zeros((N, NJ, 3), np.float32)
    w[:, :REAL, 0] = wl * (1 - r)
    w[:, :REAL, 1] = wl * r + wh * (1 - r)
    w[:, :REAL, 2] = wh * r
    idx_full = np.zeros((N, NJ), np.int64)
    idx_full[:, :REAL] = idx
    return idx_full, w


def _build_px(meta, lv):
    """L1: j = (row_sel, col_sel, py, sy, px, sx) -> 784 1-px corner gathers.
    Returns idx [N, NJ] int64, w [N, NJ] fp32."""
    N = meta['yl'].shape[0]
    W = lv['W']
    NJ, REAL = lv['NJ'], lv['REAL']
    rows = np.stack([meta['yl'], meta['yh']], axis=1)   # [N,2(rs),7,2]
    wys = np.stack([meta['hy'], meta['ly']], axis=1)
    cols = np.stack([meta['xl'], meta['xh']], axis=1)   # [N,2(cs),7,2]
    wxs = np.stack([meta['hx'], meta['lx']], axis=1)
    m = (meta['masky'][:, :, :, None, None] * meta['maskx'][:, None, None, :, :])
    row = np.broadcast_to(rows[:, :, None, :, :, None, None], (N, 2, 2, 7, 2, 7, 2))
    wy = np.broadcast_to(wys[:, :, None, :, :, None, None], (N, 2, 2, 7, 2, 7, 2)).astype(np.float32)
    col = np.broadcast_to(cols[:, None, :, None, None, :, :], (N, 2, 2, 7, 2, 7, 2))
    wx = np.broadcast_to(wxs[:, None, :, None, None, :, :], (N, 2, 2, 7, 2, 7, 2)).astype(np.float32)
    mm = np.broadcast_to(m[:, None, None], (N, 2, 2, 7, 2, 7, 2)).astype(np.float32)
    idx = (row * W + col).reshape(N, REAL)
    w = (wy * wx * mm * np.float32(0.25)).reshape(N, REAL)
    idx_full = np.zeros((N, NJ), np.int64)
    w_full = np.zeros((N, NJ), np.float32)
    idx_full[:, :REAL] = idx
    w_full[:, :REAL] = w
    return idx_full, w_full


def _build_reg(meta, lv):
    """L2/L3: bounding-region pixels + separable host-baked weights.
    Returns idx [N, NJ] int64, lhsT [N, NJ, 49] fp32."""
    N = meta['yl'].shape[0]
    H, W, WREG = lv['H'], lv['W'], lv['WREG']
    NJ, REAL = lv['NJ'], lv['REAL']
    f = np.float32
    y_base = np.floor(np.clip(meta['y'].reshape(N, -1).min(1), 0.0, H - 1)).astype(np.int64)
    x_base = np.floor(np.clip(meta['x'].reshape(N, -1).min(1), 0.0, W - 1)).astype(np.int64)
    # WY [N, WREG, 7], WX [N, WREG, 7]
    WY = np.zeros((N, WREG, POOLED), np.float32)
    WX = np.zeros((N, WREG, POOLED), np.float32)
    ridx = np.arange(N)[:, None, None]
    pidx = np.broadcast_to(np.arange(POOLED)[None, :, None], (N, POOLED, SAMP))
    np.add.at(WY, (ridx, meta['yl'] - y_base[:, None, None], pidx),
              (f(0.5) * meta['hy'] * meta['masky']).astype(np.float32))
    np.add.at(WY, (ridx, meta['yh'] - y_base[:, None, None], pidx),
              (f(0.5) * meta['ly'] * meta['masky']).astype(np.float32))
    np.add.at(WX, (ridx, meta['xl'] - x_base[:, None, None], pidx),
              (f(0.5) * meta['hx'] * meta['maskx']).astype(np.float32))
    np.add.at(WX, (ridx, meta['xh'] - x_base[:, None, None], pidx),
              (f(0.5) * meta['lx'] * meta['maskx']).astype(np.float32))
    lhsT = np.einsum('nap,nbq->nabpq', WY, WX).reshape(N, REAL, NBIN)
    dy = np.arange(WREG)
    idx = ((y_base[:, None, None] + dy[None, :, None]) * W
           + x_base[:, None, None] + dy[None, None, :]).reshape(N, REAL)
    idx_full = np.zeros((N, NJ), np.int64)
    lhsT_full = np.zeros((N, NJ, NBIN), np.float32)
    idx_full[:, :REAL] = idx
    lhsT_full[:, :REAL] = lhsT
    return idx_full, lhsT_full


def _pack_idx(jlists):
    """Pack concatenated per-group idx list [NJ_total] -> [128, NJ_total//16]
    int16 wrapped in 16 partitions, replicated 8x."""
    jl = np.asarray(jlists)
    n = jl.shape[-1]
    arr = jl.reshape(*jl.shape[:-1], n // 16, 16)   # [..., col, p]
    arr = np.swapaxes(arr, -1, -2)                  # [..., p(16), col]
    arr = np.broadcast_to(arr[..., None, :, :],
                          (*jl.shape[:-1], 8, 16, n // 16))
    return arr.reshape(*jl.shape[:-1], 128, n // 16).astype(np.int16)


def _bin_pattern(mode, NCH, REAL):
    """Fixed j->bin one-hot pattern [128, NCH, 49] for 'tri'/'px' j order."""
    NJ = NCH * 128
    j = np.arange(NJ)
    if mode == 'tri':
        # j = ((((rs*7+py)*2+sy)*7+px)*2+sx)
        px = (j // 2) % 7
        py = (j // (2 * 7 * 2)) % 7
    else:
        # j = (((((rs*2+cs)*7+py)*2+sy)*7+px)*2+sx)
        px = (j // 2) % 7
        py = (j // (2 * 7 * 2)) % 7
    bins = py * 7 + px
    pat = np.zeros((NJ, NBIN), np.float32)
    valid = j < REAL
    pat[np.arange(NJ)[valid], bins[valid]] = 1.0
    return pat.reshape(NCH, 128, NBIN).transpose(1, 0, 2)   # [128, NCH, 49]


def _host_prepare(x0, x1, x2, x3, boxes):
    """Build all per-core input tensors. Returns list of 8 dicts."""
    B = boxes.shape[0]
    feats = []
    for arr, lv, rows in ((x0, L0, F0_ROWS), (x1, L1, F1_ROWS),
                          (x2, L2, F2_ROWS), (x3, L3, F3_ROWS)):
        f = np.zeros((B, rows, C), np.float32)
        hw = lv['H'] * lv['W']
        f[:, :hw] = np.ascontiguousarray(
            np.transpose(np.asarray(arr, np.float32), (0, 2, 3, 1))).reshape(B, hw, C)
        feats.append(f)

    per_batch = []
    for b in range(B):
        bb = np.asarray(boxes[b], np.float32)
        m0 = _sample_meta(bb, L0['H'], L0['W'], L0['scale'])
        m1 = _sample_meta(bb, L1['H'], L1['W'], L1['scale'])
        m2 = _sample_meta(bb, L2['H'], L2['W'], L2['scale'])
        m3 = _sample_meta(bb, L3['H'], L3['W'], L3['scale'])
        idx0, w0 = _build_tri(m0, L0)
        idx1, w1 = _build_px(m1, L1)
        idx2, lt2 = _build_reg(m2, L2)
        idx3, lt3 = _build_reg(m3, L3)
        per_batch.append((idx0, w0, idx1, w1, idx2, lt2, idx3, lt3))

    pat0 = _bin_pattern('tri', L0['NCH'], L0['REAL'])
    pat1 = _bin_pattern('px', L1['NCH'], L1['REAL'])

    in_maps = []
    for k in range(8):
        b = k // 4
        s = (k % 4) * NROI_CORE
        idx0, w0, idx1, w1, idx2, lt2, idx3, lt3 = per_batch[b]
        sl = slice(s, s + NROI_CORE)

        cst = np.zeros((128, CST_COLS), np.float32)
        cst[:, PAT0_OFF:PAT0_OFF + 4 * NBIN] = pat0.reshape(128, -1)
        cst[:, PAT1_OFF:PAT1_OFF + 7 * NBIN] = pat1.reshape(128, -1)
        # wcol0 [128, roi*12]: col roi*12 + c*3 + slot = w0[roi, c*128+p, slot]
        wc0 = w0[sl].reshape(NROI_CORE, L0['NCH'], 128, 3)   # [roi,c,p,s]
        cst[:, WCOL0_OFF:WCOL0_OFF + NROI_CORE * 12] = (
            wc0.transpose(2, 0, 1, 3).reshape(128, -1))
        wc1 = w1[sl].reshape(NROI_CORE, L1['NCH'], 128)      # [roi,c,p]
        cst[:, WCOL1_OFF:WCOL1_OFF + NROI_CORE * 7] = (
            wc1.transpose(2, 0, 1).reshape(128, -1))
        cst[:NBIN, ID_OFF:ID_OFF + NBIN] = np.eye(NBIN, dtype=np.float32)

        idxs = np.zeros((128, IDX_COLS), np.int16)
        idxs[:, IDX0_OFF:IDX0_OFF + NGRP * IC0] = _pack_idx(
            idx0[sl].reshape(NGRP, GRP * L0['NJ'])).transpose(1, 0, 2).reshape(128, -1)
        idxs[:, IDX1_OFF:IDX1_OFF + NGRP * IC1] = _pack_idx(
            idx1[sl].reshape(NGRP, GRP * L1['NJ'])).transpose(1, 0, 2).reshape(128, -1)
        idxs[:, IDX2_OFF:IDX2_OFF + NGRP * IC2] = _pack_idx(
            idx2[sl].reshape(NGRP, GRP * L2['NJ'])).transpose(1, 0, 2).reshape(128, -1)
        idxs[:, IDX3_OFF:IDX3_OFF + NGRP * IC3] = _pack_idx(
            idx3[sl].reshape(NGRP, GRP * L3['NJ'])).transpose(1, 0, 2).reshape(128, -1)

        # lhsT k-major: lt2 [roi, NJ(=3*128), 49] -> [roi, 128, 3, 49]
        lt2k = np.ascontiguousarray(
            lt2[sl].reshape(NROI_CORE, L2['NCH'], 128, NBIN).transpose(0, 2, 1, 3))
        lt3k = np.ascontiguousarray(lt3[sl].reshape(NROI_CORE, 128, NBIN))

        in_maps.append({
            "f0": feats[0][b], "f1": feats[1][b],
            "f2": feats[2][b], "f3": feats[3][b],
            "cst": cst, "idxs": idxs, "lt2": lt2k, "lt3": lt3k,
        })
    return in_maps


def _build_module():
    from concourse import bacc, tile
    from concourse.bass import mybir
    import concourse.bass as bass_mod

    F32 = mybir.dt.float32
    I16 = mybir.dt.int16
    AP = bass_mod.AP

    nc = bacc.Bacc(None, target_bir_lowering=False)
    f0 = nc.dram_tensor("f0", [F0_ROWS, C], F32, kind="ExternalInput")
    f1 = nc.dram_tensor("f1", [F1_ROWS, C], F32, kind="ExternalInput")
    f2 = nc.dram_tensor("f2", [F2_ROWS, C], F32, kind="ExternalInput")
    f3 = nc.dram_tensor("f3", [F3_ROWS, C], F32, kind="ExternalInput")
    cst = nc.dram_tensor("cst", [128, CST_COLS], F32, kind="ExternalInput")
    idxs = nc.dram_tensor("idxs", [128, IDX_COLS], I16, kind="ExternalInput")
    lt2 = nc.dram_tensor("lt2", [NROI_CORE, 128, L2['NCH'], NBIN], F32, kind="ExternalInput")
    lt3 = nc.dram_tensor("lt3", [NROI_CORE, 128, NBIN], F32, kind="ExternalInput")
    out = nc.dram_tensor("out", [NROI_CORE, C, NBIN], F32, kind="ExternalOutput")

    # overlapping 3-px elem view of f0: stride 2px, width 3px
    f0_view = AP(f0, 0, [[2 * C, F0_ROWS // 2 - 1], [1, 3 * C]])
    gather_srcs = [f0_view, f1[:], f2[:], f3[:]]
    ELEM = [3 * C, C, C, C]
    STEP = [2 * C, C, C, C]
    ICOLS = [IC0, IC1, IC2, IC3]
    IOFF = [IDX0_OFF, IDX1_OFF, IDX2_OFF, IDX3_OFF]

    with tile.TileContext(nc) as tc:
        with (
            tc.tile_pool(name="const", bufs=1) as constp,
            tc.tile_pool(name="g0p", bufs=2) as g0p,
            tc.tile_pool(name="g1p", bufs=2) as g1p,
            tc.tile_pool(name="g2p", bufs=2) as g2p,
            tc.tile_pool(name="g3p", bufs=2) as g3p,
            tc.tile_pool(name="ltp", bufs=3) as ltp,
            tc.tile_pool(name="wp", bufs=6) as wp,
            tc.tile_pool(name="accp", bufs=4, space="PSUM") as accp,
            tc.tile_pool(name="ptp", bufs=2, space="PSUM") as ptp,
            tc.tile_pool(name="evp", bufs=3) as evp,
            tc.tile_pool(name="otp", bufs=3) as otp,
        ):
            cst_t = constp.tile([128, CST_COLS], F32)
            nc.sync.dma_start(cst_t[:], cst[:])
            idx_t = constp.tile([128, IDX_COLS], I16)
            nc.sync.dma_start(idx_t[:], idxs[:])

            gpools = [g0p, g1p, g2p, g3p]
            for grp in range(NGRP):
                gts = []
                for l, lv in enumerate(LEVELS):
                    nidx = GRP * lv['NJ']
                    gt = gpools[l].tile([128, GRP * lv['NCH'], ELEM[l]], F32,
                                        tag=f"g{l}")
                    io = IOFF[l] + grp * ICOLS[l]
                    if nidx <= 1024:
                        nc.gpsimd.dma_gather(
                            gt[:], gather_srcs[l], idx_t[:, io:io + ICOLS[l]],
                            nidx, nidx, ELEM[l], elem_step=STEP[l])
                    else:
                        # SWDGE ring cap: split into one call per ROI
                        hc = ICOLS[l] // GRP
                        for r2 in range(GRP):
                            nc.gpsimd.dma_gather(
                                gt[:, r2 * lv['NCH']:(r2 + 1) * lv['NCH'], :],
                                gather_srcs[l],
                                idx_t[:, io + r2 * hc:io + (r2 + 1) * hc],
                                lv['NJ'], lv['NJ'], ELEM[l], elem_step=STEP[l])
                    gts.append(gt)

                for r2 in range(GRP):
                    roi = grp * GRP + r2
                    lt2_t = ltp.tile([128, L2['NCH'], NBIN], F32, tag="lt2")
                    nc.sync.dma_start(lt2_t[:], lt2[roi])
                    lt3_t = ltp.tile([128, NBIN], F32, tag="lt3")
                    nc.sync.dma_start(lt3_t[:], lt3[roi])

                    acc = accp.tile([NBIN, C], F32)
                    n_mm = 12 + 7 + 3 + 1
                    mi = 0
                    # L0: 4 chunks x 3 slots
                    for c in range(L0['NCH']):
                        for s in range(3):
                            w = wp.tile([128, NBIN], F32, tag="w")
                            colw = WCOL0_OFF + roi * 12 + c * 3 + s
                            nc.vector.tensor_scalar_mul(
                                w[:],
                                cst_t[:, PAT0_OFF + c * NBIN:PAT0_OFF + (c + 1) * NBIN],
                                cst_t[:, colw:colw + 1])
                            nc.tensor.matmul(
                                acc[:], w[:],
                                gts[0][:, r2 * L0['NCH'] + c, s * C:(s + 1) * C],
                                start=(mi == 0), stop=(mi == n_mm - 1))
                            mi += 1
                    # L1: 7 chunks
                    for c in range(L1['NCH']):
                        w = wp.tile([128, NBIN], F32, tag="w")
                        colw = WCOL1_OFF + roi * 7 + c
                        nc.vector.tensor_scalar_mul(
                            w[:],
                            cst_t[:, PAT1_OFF + c * NBIN:PAT1_OFF + (c + 1) * NBIN],
                            cst_t[:, colw:colw + 1])
                        nc.tensor.matmul(
                            acc[:], w[:], gts[1][:, r2 * L1['NCH'] + c, :],
                            start=(mi == 0), stop=(mi == n_mm - 1))
                        mi += 1
                    # L2: 3 chunks, host-baked lhsT
                    for c in range(L2['NCH']):
                        nc.tensor.matmul(
                            acc[:], lt2_t[:, c, :], gts[2][:, r2 * L2['NCH'] + c, :],
                            start=(mi == 0), stop=(mi == n_mm - 1))
                        mi += 1
                    # L3: 1 chunk
                    nc.tensor.matmul(
                        acc[:], lt3_t[:], gts[3][:, r2, :],
                        start=(mi == 0), stop=(mi == n_mm - 1))
                    mi += 1

                    ev = evp.tile([NBIN, C], F32, tag="ev")
                    nc.scalar.copy(ev[:], acc[:])
                    pt = ptp.tile([128, 2, NBIN], F32, tag="pt")
                    for h in range(2):
                        nc.tensor.transpose(
                            pt[:, h, :], ev[:, h * 128:(h + 1) * 128],
                            cst_t[:NBIN, ID_OFF:ID_OFF + NBIN])
                    ot = otp.tile([128, 2, NBIN], F32, tag="ot")
                    nc.vector.tensor_copy(ot[:], pt[:])
                    # out[roi] is [256, 49]; view as [h, p, m] -> dst [p, h, m]
                    dst = out[roi].rearrange("(h p) m -> p h m", h=2)
                    nc.sync.dma_start(dst, ot[:])
    nc.finalize()
    return nc


def bench(iters=12):
    """Device-time estimate: repeat-run the compiled module with pre-staged
    device inputs (no donation). Returns (per_iter_times_s, chained_avg_s)."""
    import time
    import jax
    import numpy as np
    from jax.experimental.shard_map import shard_map
    from jax.sharding import Mesh, NamedSharding, PartitionSpec
    from concourse import bass2jax
    from concourse.bass import mybir

    nc = _MODULE_CACHE['nc']
    in_maps = LAST_RESULT['in_maps']
    bass2jax.install_neuronx_cc_hook()
    pname = nc.partition_id_tensor.name if nc.partition_id_tensor else None
    in_names, out_names, out_avals, zero_outs = [], [], [], []
    for alloc in nc.m.functions[0].allocations:
        if not isinstance(alloc, mybir.MemoryLocationSet):
            continue
        name = alloc.memorylocations[0].name
        if alloc.kind == "ExternalInput":
            if name != pname:
                in_names.append(name)
        elif alloc.kind == "ExternalOutput":
            shape = tuple(alloc.tensor_shape)
            dtype = mybir.dt.np(alloc.dtype)
            out_names.append(name)
            out_avals.append(jax.core.ShapedArray(shape, dtype))
            zero_outs.append(np.zeros(shape, dtype))
    n_params = len(in_names)
    in_names_all = in_names + out_names + ([pname] if pname else [])

    def _body(*args):
        operands = list(args)
        if pname is not None:
            operands.append(bass2jax.partition_id_tensor())
        return tuple(bass2jax._bass_exec_p.bind(
            *operands,
            out_avals=tuple(out_avals),
            in_names=tuple(in_names_all),
            out_names=tuple(out_names),
            lowering_input_output_aliases=(),
            sim_require_finite=True,
            sim_require_nnan=True,
            nc=nc,
        ))

    n_cores = 8
    devices = jax.devices()[:n_cores]
    mesh = Mesh(np.asarray(devices), ("core",))
    nio = n_params + len(out_names)
    fn = jax.jit(
        shard_map(_body, mesh=mesh, in_specs=(PartitionSpec("core"),) * nio,
                  out_specs=(PartitionSpec("core"),) * len(out_names),
                  check_rep=False),
        keep_unused=True)
    per_core = [[np.asarray(m[name]) for name in in_names] for m in in_maps]
    concat_in = [np.concatenate([per_core[c][i] for c in range(n_cores)], axis=0)
                 for i in range(n_params)]
    concat_zeros = [np.zeros((n_cores * z.shape[0], *z.shape[1:]), z.dtype)
                    for z in zero_outs]
    shard = NamedSharding(mesh, PartitionSpec("core"))
    dev_in = [jax.device_put(a, shard) for a in concat_in + concat_zeros]
    outs = fn(*dev_in)
    jax.block_until_ready(outs)
    times = []
    for _ in range(iters):
        t0 = time.perf_counter()
        outs = fn(*dev_in)
        jax.block_until_ready(outs)
        times.append(time.perf_counter() - t0)
    t0 = time.perf_counter()
    outs_list = [fn(*dev_in) for _ in range(iters)]
    jax.block_until_ready(outs_list)
    chained = (time.perf_counter() - t0) / iters
    return times, chained


def kernel(x0, x1, x2, x3, boxes):
    from concourse.bass_utils import run_bass_kernel_spmd
    in_maps = _host_prepare(x0, x1, x2, x3, boxes)
    if 'nc' not in _MODULE_CACHE:
        _MODULE_CACHE['nc'] = _build_module()
    nc = _MODULE_CACHE['nc']
    res = run_bass_kernel_spmd(nc, in_maps, list(range(8)), trace=TRACE)
    LAST_RESULT['res'] = res
    LAST_RESULT['in_maps'] = in_maps
    outs = [res.results[k]["out"] for k in range(8)]
    full = np.concatenate(outs, axis=0)           # [1024, 256, 49]
    return full.reshape(1024, C, POOLED, POOLED).astype(np.float32)



# revision 44
# speedup vs baseline: 1.0058x; 1.0058x over previous
"""Multi-level ROI Align (FPN pooler, 4 levels summed) on 8 Trainium2 cores.

Strategy: shard ROIs across cores (core k: batch k//4, 128 ROIs). All gather
indices and bilinear weights are computed on host from `boxes`; the device
kernel does the heavy lifting: HBM pixel gathers (dma_gather) + weighted
scatter-reduction into 7x7 bins via PSUM-accumulating matmuls.

Per ROI, per level:
  out[bin, c] = sum_j W[j, bin] * G[j, c]
where G rows are gathered 2-px vectors (C=256/px) and W is built on device as
fixed_pattern * per-partition scalar (L0/L1) or host-baked dense (L2/L3).

v6:
- bf16 features/weights/output (fp32 PSUM accumulation).
- gather calls are block-batched (the Q7 SWDGE has ~1us fixed cost/call) but
  each call stays <= 1024 descriptors (hard SWDGE ring-carveout cap,
  empirically 1024 ok / 1536 hangs).
- L0 is split into even-row / odd-row feature copies: every bilinear sample
  reads one even and one odd row, so each ROI contributes exactly 196
  2-px elems per parity (idx fits int16 at 1-px granularity), padded to 256
  for per-ROI chunk purity. 8 matmuls/ROI, no 3-px overfetch.
- L1: 2-px elems, 392 j's = 384 in block calls + 8 in one upfront tail call
  (tail chunks hold 16 ROIs; 16 static one-hot patterns, bins fixed).
- L2/L3: region pixels, 324/100 padded with idx=0 to 384/128.
- all 20+2 weight matrices of a ROI are built by 2 DVE tensor_tensor ops
  (pattern blocks * wcol columns broadcast via a 0-stride AP).
- output written [roi, 49, 256] bf16; host transposes + casts to fp32.
"""
import os
import sys
import numpy as np
import ml_dtypes

sys.path.insert(0, '/opt/trn_rl_repo')

BF16 = ml_dtypes.bfloat16

POOLED = 7
SAMP = 2
NBIN = 49
C = 256
IMG = 800.0

NSAMP = 196         # samples per ROI (7x2 x 7x2)
L0P = 256           # padded per-parity list length (2 chunks)
REAL = 392          # L1 j's per ROI (2 rowsel x 196)
MAIN = 384          # L1 j's in block calls
TAIL = 8            # L1 j's in the shared tail call
TAIL_BINS = [45, 45, 46, 46, 47, 47, 48, 48]   # bins of L1 j 384..391

L0 = dict(H=200, W=200, scale=0.25)
L1 = dict(H=100, W=100, scale=0.125)
L2 = dict(H=50, W=50, scale=0.0625, NJ=384, REAL=324, NCH=3, WREG=18)
L3 = dict(H=25, W=25, scale=0.03125, NJ=128, REAL=100, NCH=1, WREG=10)

NROI_CORE = 128     # ROIs per core
BLK = 4             # ROIs per gather-call block
NBLK = NROI_CORE // BLK

# padded flat pixel counts of the feature buffers
F0P_ROWS = 20004    # per-parity f0 (100 rows x 200 px + 2-px overrun)
F1_ROWS = 10004     # covers 2-px elem overrun
F2_ROWS = 3400      # covers region overrun (y,x up to 66)
F3_ROWS = 900       # covers region overrun (y,x up to 33)

# const bf16 column layout (per partition): pre-tiled pattern blocks.
#   PATWM [14, 49]: k<8 -> PATP[:, (k%4)//2] (L0: par*4+c*2+s)
#                   k 8..13 -> PATM[:, (k-8)//2] (L1 main c*2+s)
#   PATWT [16, 2, 49]: variant r = roi%16, 2 copies of PATT_r (L1 tail)
PATWM_OFF = 0
PATWT_OFF = PATWM_OFF + 14 * NBIN
CST_COLS = PATWT_OFF + 16 * 2 * NBIN
# bf16 per-ROI scalar weight columns, 16 per roi:
#   0..7   L0 (par*4 + c*2 + s), 8..13 L1 main (c*2+s)  [14 "main" cols]
#   14..15 L1 tail (s)                                  [2 "tail" cols]
WPR = 16
WCOL_COLS = NROI_CORE * WPR

# idx int16 column layout (per partition)
IC0 = BLK * L0P // 16       # 64 cols per block per parity
IC1 = BLK * MAIN // 16      # 96
IC2 = BLK * L2['NJ'] // 16  # 96
IC3 = BLK * L3['NJ'] // 16  # 32
ICT = NROI_CORE * TAIL // 16    # 64 cols, L1 tail list
IDX0E_OFF = 0
IDX0O_OFF = IDX0E_OFF + NBLK * IC0
IDX1_OFF = IDX0O_OFF + NBLK * IC0
IDX2_OFF = IDX1_OFF + NBLK * IC1
IDX3_OFF = IDX2_OFF + NBLK * IC2
IDXT1_OFF = IDX3_OFF + NBLK * IC3
IDX_COLS = IDXT1_OFF + ICT

_MODULE_CACHE = {}
TRACE = False
LAST_RESULT = {}


def _sample_meta(boxes_b, H, W, scale):
    """Per-ROI sample geometry in fp32, matching reference op order.
    boxes_b: [N, 4] fp32. Returns dict of [N,7,2] arrays."""
    f = np.float32
    b = boxes_b.astype(np.float32)
    x1 = b[:, 0] * f(scale)
    y1 = b[:, 1] * f(scale)
    x2 = b[:, 2] * f(scale)
    y2 = b[:, 3] * f(scale)
    rw = np.maximum(x2 - x1, f(1.0))
    rh = np.maximum(y2 - y1, f(1.0))
    bw = rw / f(POOLED)
    bh = rh / f(POOLED)
    g = (np.arange(POOLED, dtype=np.float32)[:, None]
         + (np.arange(SAMP, dtype=np.float32)[None, :] + f(0.5)) / f(SAMP))
    y = y1[:, None, None] + g[None] * bh[:, None, None]   # [N,7,2]
    x = x1[:, None, None] + g[None] * bw[:, None, None]
    masky = ((y >= f(-1.0)) & (y <= f(H))).astype(np.float32)
    maskx = ((x >= f(-1.0)) & (x <= f(W))).astype(np.float32)
    yc = np.clip(y, f(0.0), f(H - 1))
    xc = np.clip(x, f(0.0), f(W - 1))
    yl = np.floor(yc).astype(np.int64)
    xl = np.floor(xc).astype(np.int64)
    yh = np.minimum(yl + 1, H - 1)
    xh = np.minimum(xl + 1, W - 1)
    ly = (yc - yl.astype(np.float32)).astype(np.float32)
    lx = (xc - xl.astype(np.float32)).astype(np.float32)
    hy = (f(1.0) - ly).astype(np.float32)
    hx = (f(1.0) - lx).astype(np.float32)
    return dict(yl=yl, yh=yh, xl=xl, xh=xh, ly=ly, lx=lx, hy=hy, hx=hx,
                masky=masky, maskx=maskx, x=x, y=y)


def _build_l0_parity(meta, lv):
    """L0 even/odd-row split. Per parity: 196 2-px elems in (py,sy,px,sx)
    order. Returns (idx_e, w_e, idx_o, w_o): idx [N,196] in parity-local px
    units, w [N,196,2]."""
    N = meta['yl'].shape[0]
    H, W = lv['H'], lv['W']
    sh = (N, 7, 2, 7, 2)
    yl = meta['yl']
    even = (yl % 2 == 0)
    # even-row: yl itself when even, else yl+1 (clamped: yl=H-1 odd -> weight
    # is ly=0 exactly, point at yl-1 harmlessly)
    ye = np.where(even, yl, np.where(yl == H - 1, yl - 1, yl + 1))
    yo = np.where(even, yl + 1, yl)
    wy_e = np.where(even, meta['hy'], meta['ly']) * meta['masky']
    wy_o = np.where(even, meta['ly'], meta['hy']) * meta['masky']

    def expand(yv, wyv):
        row = np.broadcast_to(yv[:, :, :, None, None], sh)
        wy = np.broadcast_to(wyv[:, :, :, None, None], sh).astype(np.float32)
        xl = np.broadcast_to(meta['xl'][:, None, None, :, :], sh)
        hx = np.broadcast_to(meta['hx'][:, None, None, :, :], sh).astype(np.float32)
        lx = np.broadcast_to(meta['lx'][:, None, None, :, :], sh).astype(np.float32)
        mx = np.broadcast_to(meta['maskx'][:, None, None, :, :], sh).astype(np.float32)
        idx = ((row >> 1) * W + xl).reshape(N, NSAMP)
        w = np.zeros((N, NSAMP, 2), np.float32)
        w[:, :, 0] = (wy * hx * mx * np.float32(0.25)).reshape(N, NSAMP)
        w[:, :, 1] = (wy * lx * mx * np.float32(0.25)).reshape(N, NSAMP)
        return idx, w

    idx_e, w_e = expand(ye, wy_e)
    idx_o, w_o = expand(yo, wy_o)
    return idx_e, w_e, idx_o, w_o


def _build_px2(meta, lv):
    """L1: 2-px elems, j=(rs,py,sy,px,sx). idx [N,392], w [N,392,2]."""
    N = meta['yl'].shape[0]
    W = lv['W']
    sh = (N, 2, 7, 2, 7, 2)
    rows = np.stack([meta['yl'], meta['yh']], axis=1)          # [N,2,7,2]
    wys = np.stack([meta['hy'], meta['ly']], axis=1)
    m = (meta['masky'][:, :, :, None, None] * meta['maskx'][:, None, None, :, :])
    row = np.broadcast_to(rows[:, :, :, :, None, None], sh)
    wy = np.broadcast_to(wys[:, :, :, :, None, None], sh).astype(np.float32)
    xl = np.broadcast_to(meta['xl'][:, None, None, None, :, :], sh)
    hx = np.broadcast_to(meta['hx'][:, None, None, None, :, :], sh).astype(np.float32)
    lx = np.broadcast_to(meta['lx'][:, None, None, None, :, :], sh).astype(np.float32)
    mm = np.broadcast_to(m[:, None], sh).astype(np.float32)
    flat = (row * W + xl).reshape(N, REAL)
    w = np.zeros((N, REAL, 2), np.float32)
    w[:, :, 0] = (wy * hx * mm * np.float32(0.25)).reshape(N, REAL)
    w[:, :, 1] = (wy * lx * mm * np.float32(0.25)).reshape(N, REAL)
    return flat, w


def _build_reg(meta, lv):
    """L2/L3: bounding-region pixels + separable host-baked weights.
    Returns idx [N, NJ] int64 (pad idx=0), lhsT [N, NJ, 49] fp32."""
    N = meta['yl'].shape[0]
    H, W, WREG = lv['H'], lv['W'], lv['WREG']
    NJ, RL = lv['NJ'], lv['REAL']
    f = np.float32
    y_base = np.floor(np.clip(meta['y'].reshape(N, -1).min(1), 0.0, H - 1)).astype(np.int64)
    x_base = np.floor(np.clip(meta['x'].reshape(N, -1).min(1), 0.0, W - 1)).astype(np.int64)
    WY = np.zeros((N, WREG, POOLED), np.float32)
    WX = np.zeros((N, WREG, POOLED), np.float32)
    ridx = np.arange(N)[:, None, None]
    pidx = np.broadcast_to(np.arange(POOLED)[None, :, None], (N, POOLED, SAMP))
    np.add.at(WY, (ridx, meta['yl'] - y_base[:, None, None], pidx),
              (f(0.5) * meta['hy'] * meta['masky']).astype(np.float32))
    np.add.at(WY, (ridx, meta['yh'] - y_base[:, None, None], pidx),
              (f(0.5) * meta['ly'] * meta['masky']).astype(np.float32))
    np.add.at(WX, (ridx, meta['xl'] - x_base[:, None, None], pidx),
              (f(0.5) * meta['hx'] * meta['maskx']).astype(np.float32))
    np.add.at(WX, (ridx, meta['xh'] - x_base[:, None, None], pidx),
              (f(0.5) * meta['lx'] * meta['maskx']).astype(np.float32))
    lhsT = np.einsum('nap,nbq->nabpq', WY, WX).reshape(N, RL, NBIN)
    dy = np.arange(WREG)
    idx = ((y_base[:, None, None] + dy[None, :, None]) * W
           + x_base[:, None, None] + dy[None, None, :]).reshape(N, RL)
    idx_full = np.zeros((N, NJ), np.int64)
    lhsT_full = np.zeros((N, NJ, NBIN), np.float32)
    idx_full[:, :RL] = idx
    lhsT_full[:, :RL] = lhsT
    return idx_full, lhsT_full


def _pack_idx(jlists):
    """Pack idx list [..., NJ_total] -> [..., 128, NJ_total//16] int16
    wrapped in 16 partitions, replicated 8x."""
    jl = np.asarray(jlists)
    n = jl.shape[-1]
    arr = jl.reshape(*jl.shape[:-1], n // 16, 16)   # [..., col, p]
    arr = np.swapaxes(arr, -1, -2)                  # [..., p(16), col]
    arr = np.broadcast_to(arr[..., None, :, :],
                          (*jl.shape[:-1], 8, 16, n // 16))
    return arr.reshape(*jl.shape[:-1], 128, n // 16).astype(np.int16)


def _patterns():
    """PATP [128, 2, 49]: L0 parity-sample pattern (k=(py,sy,px,sx) order,
    196 real). PATM [128, 3, 49]: L1 main (j 0..383 of the 392-order).
    PATT [128, 16, 49]: L1 tail variants."""
    k = np.arange(2 * 128)
    px = (k // 2) % 7
    py = k // 28
    patp = np.zeros((2 * 128, NBIN), np.float32)
    v = k < NSAMP
    patp[np.arange(2 * 128)[v], (py * 7 + px)[v]] = 1.0
    patp = patp.reshape(2, 128, NBIN).transpose(1, 0, 2)

    j = np.arange(MAIN)
    px = (j // 2) % 7
    py = (j // 28) % 7
    patm = np.zeros((MAIN, NBIN), np.float32)
    patm[np.arange(MAIN), py * 7 + px] = 1.0
    patm = patm.reshape(3, 128, NBIN).transpose(1, 0, 2)

    patt = np.zeros((128, 16, NBIN), np.float32)
    for r in range(16):
        for kk in range(TAIL):
            patt[r * TAIL + kk, r, TAIL_BINS[kk]] = 1.0
    return patp, patm, patt


def _host_prepare(x0, x1, x2, x3, boxes):
    """Build all per-core input tensors. Returns list of 8 dicts."""
    B = boxes.shape[0]
    # f0 split by row parity
    f0b = np.transpose(np.asarray(x0, np.float32), (0, 2, 3, 1))   # [B,H,W,C]
    f0e = np.zeros((B, F0P_ROWS, C), BF16)
    f0o = np.zeros((B, F0P_ROWS, C), BF16)
    f0e[:, :100 * 200] = f0b[:, 0::2].reshape(B, -1, C).astype(BF16)
    f0o[:, :100 * 200] = f0b[:, 1::2].reshape(B, -1, C).astype(BF16)
    feats = [f0e, f0o]
    for arr, lv, rows in ((x1, L1, F1_ROWS), (x2, L2, F2_ROWS), (x3, L3, F3_ROWS)):
        f = np.zeros((B, rows, C), BF16)
        hw = lv['H'] * lv['W']
        f[:, :hw] = np.ascontiguousarray(
            np.transpose(np.asarray(arr, np.float32), (0, 2, 3, 1))
        ).reshape(B, hw, C).astype(BF16)
        feats.append(f)

    per_batch = []
    for b in range(B):
        bb = np.asarray(boxes[b], np.float32)
        m0 = _sample_meta(bb, L0['H'], L0['W'], L0['scale'])
        m1 = _sample_meta(bb, L1['H'], L1['W'], L1['scale'])
        m2 = _sample_meta(bb, L2['H'], L2['W'], L2['scale'])
        m3 = _sample_meta(bb, L3['H'], L3['W'], L3['scale'])
        i0e, w0e, i0o, w0o = _build_l0_parity(m0, L0)
        idx1, w1 = _build_px2(m1, L1)
        idx2, lt2 = _build_reg(m2, L2)
        idx3, lt3 = _build_reg(m3, L3)
        per_batch.append((i0e, w0e, i0o, w0o, idx1, w1, idx2, lt2, idx3, lt3))

    patp, patm, patt = _patterns()

    in_maps = []
    for k in range(8):
        b = k // 4
        s = (k % 4) * NROI_CORE
        i0e, w0e, i0o, w0o, idx1, w1, idx2, lt2, idx3, lt3 = per_batch[b]
        sl = slice(s, s + NROI_CORE)

        cst = np.zeros((128, CST_COLS), BF16)
        patwm = np.concatenate(
            [patp[:, [(k_ % 4) // 2 for k_ in range(8)], :],
             patm[:, [k_ // 2 for k_ in range(6)], :]], axis=1)  # [128,14,49]
        cst[:, PATWM_OFF:PATWM_OFF + 14 * NBIN] = patwm.reshape(128, -1).astype(BF16)
        patwt = np.broadcast_to(patt[:, :, None, :], (128, 16, 2, NBIN))
        cst[:, PATWT_OFF:PATWT_OFF + 16 * 2 * NBIN] = patwt.reshape(128, -1).astype(BF16)

        # wcol [128, roi*16 + k]
        wcol = np.zeros((128, WCOL_COLS), np.float32)
        cols = np.arange(NROI_CORE) * WPR
        # L0: pad [N,196,2] -> [N,256,2]; col par*4 + c*2 + s
        for par, warr in ((0, w0e[sl]), (1, w0o[sl])):
            wp_ = np.zeros((NROI_CORE, L0P, 2), np.float32)
            wp_[:, :NSAMP] = warr
            wp_ = wp_.reshape(NROI_CORE, 2, 128, 2)          # [roi, c, p, s]
            for c in range(2):
                for s2 in range(2):
                    wcol[:, cols + par * 4 + c * 2 + s2] = wp_[:, c, :, s2].T
        # L1 main
        w1c = w1[sl]
        wm1 = w1c[:, :MAIN].reshape(NROI_CORE, 3, 128, 2)
        for c in range(3):
            for s2 in range(2):
                wcol[:, cols + 8 + c * 2 + s2] = wm1[:, c, :, s2].T
        # L1 tail: value at partition (roi%16)*8 + kk
        prt = (np.arange(NROI_CORE) % 16)[:, None] * TAIL + np.arange(TAIL)[None, :]
        for s2 in range(2):
            wt_ = np.zeros((NROI_CORE, 128), np.float32)
            np.put_along_axis(wt_, prt, w1c[:, MAIN:, s2], axis=1)
            wcol[:, cols + 14 + s2] = wt_.T
        wcol = wcol.astype(BF16)

        # idx lists
        def padl(a, n):
            out = np.zeros((a.shape[0], n), np.int64)
            out[:, :a.shape[1]] = a
            return out

        idxs = np.zeros((128, IDX_COLS), np.int16)
        idxs[:, IDX0E_OFF:IDX0E_OFF + NBLK * IC0] = _pack_idx(
            padl(i0e[sl], L0P).reshape(NBLK, BLK * L0P)
        ).transpose(1, 0, 2).reshape(128, -1)
        idxs[:, IDX0O_OFF:IDX0O_OFF + NBLK * IC0] = _pack_idx(
            padl(i0o[sl], L0P).reshape(NBLK, BLK * L0P)
        ).transpose(1, 0, 2).reshape(128, -1)
        idxs[:, IDX1_OFF:IDX1_OFF + NBLK * IC1] = _pack_idx(
            idx1[sl][:, :MAIN].reshape(NBLK, BLK * MAIN)
        ).transpose(1, 0, 2).reshape(128, -1)
        idxs[:, IDX2_OFF:IDX2_OFF + NBLK * IC2] = _pack_idx(
            idx2[sl].reshape(NBLK, BLK * L2['NJ'])
        ).transpose(1, 0, 2).reshape(128, -1)
        idxs[:, IDX3_OFF:IDX3_OFF + NBLK * IC3] = _pack_idx(
            idx3[sl].reshape(NBLK, BLK * L3['NJ'])
        ).transpose(1, 0, 2).reshape(128, -1)
        idxs[:, IDXT1_OFF:IDXT1_OFF + ICT] = _pack_idx(
            idx1[sl][:, MAIN:].reshape(NROI_CORE * TAIL))

        # lhsT k-major: lt2 [roi, 3*128, 49] -> [roi, 128, 3, 49];
        # lt3 [roi, 128, 49]; combined [roi, 128, 4, 49] bf16
        lt2k = lt2[sl].reshape(NROI_CORE, L2['NCH'], 128, NBIN).transpose(0, 2, 1, 3)
        lt3k = lt3[sl].reshape(NROI_CORE, 1, 128, NBIN).transpose(0, 2, 1, 3)
        lt = np.ascontiguousarray(
            np.concatenate([lt2k, lt3k], axis=2)).astype(BF16)

        in_maps.append({
            "f0e": feats[0][b], "f0o": feats[1][b], "f1": feats[2][b],
            "f2": feats[3][b], "f3": feats[4][b],
            "cst": cst, "wcol": wcol, "idxs": idxs, "lt": lt,
        })
    return in_maps


def _build_module():
    from concourse import bacc, tile
    from concourse.bass import mybir
    import concourse.bass as bass_mod

    F32 = mybir.dt.float32
    BF = mybir.dt.bfloat16
    I16 = mybir.dt.int16
    AP = bass_mod.AP

    nc = bacc.Bacc(None, target_bir_lowering=False)
    f0e = nc.dram_tensor("f0e", [F0P_ROWS, C], BF, kind="ExternalInput")
    f0o = nc.dram_tensor("f0o", [F0P_ROWS, C], BF, kind="ExternalInput")
    f1 = nc.dram_tensor("f1", [F1_ROWS, C], BF, kind="ExternalInput")
    f2 = nc.dram_tensor("f2", [F2_ROWS, C], BF, kind="ExternalInput")
    f3 = nc.dram_tensor("f3", [F3_ROWS, C], BF, kind="ExternalInput")
    cst = nc.dram_tensor("cst", [128, CST_COLS], BF, kind="ExternalInput")
    wcol = nc.dram_tensor("wcol", [128, WCOL_COLS], BF, kind="ExternalInput")
    idxs = nc.dram_tensor("idxs", [128, IDX_COLS], I16, kind="ExternalInput")
    lt = nc.dram_tensor("lt", [NROI_CORE, 128, 4, NBIN], BF, kind="ExternalInput")
    out = nc.dram_tensor("out", [NROI_CORE, NBIN, C], BF, kind="ExternalOutput")

    # 2-px-elem views at 1-px stride
    f0eV = AP(f0e, 0, [[C, F0P_ROWS - 2], [1, 2 * C]])
    f0oV = AP(f0o, 0, [[C, F0P_ROWS - 2], [1, 2 * C]])
    f1v = AP(f1, 0, [[C, F1_ROWS - 2], [1, 2 * C]])

    with tile.TileContext(nc) as tc:
        with (
            tc.tile_pool(name="const", bufs=1) as constp,
            tc.tile_pool(name="g0ep", bufs=2) as g0ep,
            tc.tile_pool(name="g0op", bufs=2) as g0op,
            tc.tile_pool(name="g1p", bufs=2) as g1p,
            tc.tile_pool(name="g2p", bufs=2) as g2p,
            tc.tile_pool(name="g3p", bufs=2) as g3p,
            tc.tile_pool(name="ltp", bufs=3) as ltp,
            tc.tile_pool(name="wp", bufs=4) as wp,
            tc.tile_pool(name="accp", bufs=4, space="PSUM") as accp,
            tc.tile_pool(name="evp", bufs=3) as evp,
        ):
            cst_t = constp.tile([128, CST_COLS], BF)
            nc.sync.dma_start(cst_t[:], cst[:])
            wcol_t = constp.tile([128, WCOL_COLS], BF, tag="wcol")
            nc.sync.dma_start(wcol_t[:], wcol[:])
            idx_t = constp.tile([128, IDX_COLS], I16)
            nc.sync.dma_start(idx_t[:], idxs[:])

            # upfront L1 tail gather: all 128 ROIs' last-8 j's
            g1T = constp.tile([128, NROI_CORE // 16, 2 * C], BF, tag="g1T")
            nc.gpsimd.dma_gather(
                g1T[:], f1v, idx_t[:, IDXT1_OFF:IDXT1_OFF + ICT],
                NROI_CORE * TAIL, NROI_CORE * TAIL, 2 * C, elem_step=C)

            for blk in range(NBLK):
                # L0 parity gathers: BLK*256 = 1024 descs each (at the cap)
                gt0e = g0ep.tile([128, BLK * 2, 2 * C], BF, tag="g0e")
                io = IDX0E_OFF + blk * IC0
                nc.gpsimd.dma_gather(
                    gt0e[:], f0eV, idx_t[:, io:io + IC0],
                    BLK * L0P, BLK * L0P, 2 * C, elem_step=C)
                gt0o = g0op.tile([128, BLK * 2, 2 * C], BF, tag="g0o")
                io = IDX0O_OFF + blk * IC0
                nc.gpsimd.dma_gather(
                    gt0o[:], f0oV, idx_t[:, io:io + IC0],
                    BLK * L0P, BLK * L0P, 2 * C, elem_step=C)
                # L1/L2: 2-ROI halves keep calls at 768 descs
                gt1 = g1p.tile([128, BLK * 3, 2 * C], BF, tag="g1")
                io = IDX1_OFF + blk * IC1
                for h in range(2):
                    nc.gpsimd.dma_gather(
                        gt1[:, h * 6:(h + 1) * 6, :], f1v,
                        idx_t[:, io + h * 48:io + (h + 1) * 48],
                        2 * MAIN, 2 * MAIN, 2 * C, elem_step=C)
                gt2 = g2p.tile([128, BLK * 3, C], BF, tag="g2")
                io = IDX2_OFF + blk * IC2
                for h in range(2):
                    nc.gpsimd.dma_gather(
                        gt2[:, h * 6:(h + 1) * 6, :], f2[:],
                        idx_t[:, io + h * 48:io + (h + 1) * 48],
                        2 * L2['NJ'], 2 * L2['NJ'], C, elem_step=C)
                gt3 = g3p.tile([128, BLK, C], BF, tag="g3")
                io = IDX3_OFF + blk * IC3
                nc.gpsimd.dma_gather(
                    gt3[:], f3[:], idx_t[:, io:io + IC3],
                    BLK * L3['NJ'], BLK * L3['NJ'], C, elem_step=C)

                for bri in range(BLK):
                    roi = blk * BLK + bri
                    tr = roi % 16            # tail pattern index
                    tch = roi // 16          # tail chunk
                    lt_t = ltp.tile([128, 4, NBIN], BF, tag="lt")
                    nc.sync.dma_start(lt_t[:], lt[roi])

                    acc = accp.tile([NBIN, C], F32)
                    n_mm = 8 + 8 + 3 + 1
                    mi = 0
                    wb = roi * WPR

                    # build all weight mats in 2 DVE ops: pattern blocks *
                    # wcol columns broadcast (0-stride) across the 49 bins
                    wm = wp.tile([128, 14, NBIN], BF, tag="wm")
                    wsl = wcol_t[:, wb:wb + 14]
                    nc.vector.tensor_mul(
                        wm[:],
                        cst_t[:, PATWM_OFF:PATWM_OFF + 14 * NBIN].rearrange(
                            "p (k b) -> p k b", b=NBIN),
                        AP(wsl.tensor, wsl.offset, [*wsl.ap, [0, NBIN]]))
                    wt = wp.tile([128, 2, NBIN], BF, tag="wt")
                    tsl = wcol_t[:, wb + 14:wb + 16]
                    to = PATWT_OFF + tr * 2 * NBIN
                    nc.vector.tensor_mul(
                        wt[:],
                        cst_t[:, to:to + 2 * NBIN].rearrange(
                            "p (k b) -> p k b", b=NBIN),
                        AP(tsl.tensor, tsl.offset, [*tsl.ap, [0, NBIN]]))

                    # L0: 2 parities x 2 chunks x 2 slots
                    for par, gt0 in ((0, gt0e), (1, gt0o)):
                        for c in range(2):
                            for s2 in range(2):
                                nc.tensor.matmul(
                                    acc[:], wm[:, par * 4 + c * 2 + s2, :],
                                    gt0[:, bri * 2 + c, s2 * C:(s2 + 1) * C],
                                    start=(mi == 0), stop=(mi == n_mm - 1))
                                mi += 1
                    # L1 main: 3 chunks x 2 slots
                    for c in range(3):
                        for s2 in range(2):
                            nc.tensor.matmul(
                                acc[:], wm[:, 8 + c * 2 + s2, :],
                                gt1[:, bri * 3 + c, s2 * C:(s2 + 1) * C],
                                start=(mi == 0), stop=(mi == n_mm - 1))
                            mi += 1
                    # L1 tail: 2 slots
                    for s2 in range(2):
                        nc.tensor.matmul(
                            acc[:], wt[:, s2, :], g1T[:, tch, s2 * C:(s2 + 1) * C],
                            start=(mi == 0), stop=(mi == n_mm - 1))
                        mi += 1
                    # L2: 3 chunks, host-baked lhsT
                    for c in range(3):
                        nc.tensor.matmul(
                            acc[:], lt_t[:, c, :], gt2[:, bri * 3 + c, :],
                            start=(mi == 0), stop=(mi == n_mm - 1))
                        mi += 1
                    # L3: 1 chunk
                    nc.tensor.matmul(
                        acc[:], lt_t[:, 3, :], gt3[:, bri, :],
                        start=(mi == 0), stop=(mi == n_mm - 1))
                    mi += 1

                    ev = evp.tile([NBIN, C], BF, tag="ev")
                    nc.scalar.copy(ev[:], acc[:])
                    nc.sync.dma_start(out[roi], ev[:])
    nc.finalize()
    return nc


def bench(iters=12):
    """Device-time estimate: repeat-run the compiled module with pre-staged
    device inputs (no donation). Returns (per_iter_times_s, chained_avg_s)."""
    import time
    import jax
    from jax.experimental.shard_map import shard_map
    from jax.sharding import Mesh, NamedSharding, PartitionSpec
    from concourse import bass2jax
    from concourse.bass import mybir

    nc = _MODULE_CACHE['nc']
    in_maps = LAST_RESULT['in_maps']
    bass2jax.install_neuronx_cc_hook()
    pname = nc.partition_id_tensor.name if nc.partition_id_tensor else None
    in_names, out_names, out_avals, zero_outs = [], [], [], []
    for alloc in nc.m.functions[0].allocations:
        if not isinstance(alloc, mybir.MemoryLocationSet):
            continue
        name = alloc.memorylocations[0].name
        if alloc.kind == "ExternalInput":
            if name != pname:
                in_names.append(name)
        elif alloc.kind == "ExternalOutput":
            shape = tuple(alloc.tensor_shape)
            dtype = mybir.dt.np(alloc.dtype)
            out_names.append(name)
            out_avals.append(jax.core.ShapedArray(shape, dtype))
            zero_outs.append(np.zeros(shape, dtype))
    n_params = len(in_names)
    in_names_all = in_names + out_names + ([pname] if pname else [])

    def _body(*args):
        operands = list(args)
        if pname is not None:
            operands.append(bass2jax.partition_id_tensor())
        return tuple(bass2jax._bass_exec_p.bind(
            *operands,
            out_avals=tuple(out_avals),
            in_names=tuple(in_names_all),
            out_names=tuple(out_names),
            lowering_input_output_aliases=(),
            sim_require_finite=True,
            sim_require_nnan=True,
            nc=nc,
        ))

    n_cores = 8
    devices = jax.devices()[:n_cores]
    mesh = Mesh(np.asarray(devices), ("core",))
    nio = n_params + len(out_names)
    fn = jax.jit(
        shard_map(_body, mesh=mesh, in_specs=(PartitionSpec("core"),) * nio,
                  out_specs=(PartitionSpec("core"),) * len(out_names),
                  check_rep=False),
        keep_unused=True)
    per_core = [[np.asarray(m[name]) for name in in_names] for m in in_maps]
    concat_in = [np.concatenate([per_core[c][i] for c in range(n_cores)], axis=0)
                 for i in range(n_params)]
    concat_zeros = [np.zeros((n_cores * z.shape[0], *z.shape[1:]), z.dtype)
                    for z in zero_outs]
    shard = NamedSharding(mesh, PartitionSpec("core"))
    dev_in = [jax.device_put(a, shard) for a in concat_in + concat_zeros]
    outs = fn(*dev_in)
    jax.block_until_ready(outs)
    times = []
    for _ in range(iters):
        t0 = time.perf_counter()
        outs = fn(*dev_in)
        jax.block_until_ready(outs)
        times.append(time.perf_counter() - t0)
    t0 = time.perf_counter()
    outs_list = [fn(*dev_in) for _ in range(iters)]
    jax.block_until_ready(outs_list)
    chained = (time.perf_counter() - t0) / iters
    return times, chained


def bench_floor(iters=12):
    """Chained-dispatch floor: same bench loop on a trivial copy kernel.
    Subtracting this from bench()'s chained average isolates device time."""
    import time
    import jax
    from jax.experimental.shard_map import shard_map
    from jax.sharding import Mesh, NamedSharding, PartitionSpec
    from concourse import bacc, tile, bass2jax
    from concourse.bass import mybir

    if 'tiny' not in _MODULE_CACHE:
        F32 = mybir.dt.float32
        nct = bacc.Bacc(None, target_bir_lowering=False)
        xin = nct.dram_tensor("xin", [128, 128], F32, kind="ExternalInput")
        xout = nct.dram_tensor("xout", [128, 128], F32, kind="ExternalOutput")
        with tile.TileContext(nct) as tc:
            with tc.tile_pool(name="p", bufs=1) as p:
                t = p.tile([128, 128], F32)
                nct.sync.dma_start(t[:], xin[:])
                nct.sync.dma_start(xout[:], t[:])
        nct.finalize()
        _MODULE_CACHE['tiny'] = nct
    nct = _MODULE_CACHE['tiny']
    bass2jax.install_neuronx_cc_hook()
    pname = nct.partition_id_tensor.name if nct.partition_id_tensor else None
    out_avals = [jax.core.ShapedArray((128, 128), np.float32)]

    def _body(*args):
        operands = list(args)
        if pname is not None:
            operands.append(bass2jax.partition_id_tensor())
        return tuple(bass2jax._bass_exec_p.bind(
            *operands,
            out_avals=tuple(out_avals),
            in_names=("xin", "xout") + ((pname,) if pname else ()),
            out_names=("xout",),
            lowering_input_output_aliases=(),
            sim_require_finite=True,
            sim_require_nnan=True,
            nc=nct,
        ))

    n_cores = 8
    devices = jax.devices()[:n_cores]
    mesh = Mesh(np.asarray(devices), ("core",))
    fn = jax.jit(
        shard_map(_body, mesh=mesh, in_specs=(PartitionSpec("core"),) * 2,
                  out_specs=(PartitionSpec("core"),), check_rep=False),
        keep_unused=True)
    shard = NamedSharding(mesh, PartitionSpec("core"))
    dev_in = [jax.device_put(np.zeros((n_cores * 128, 128), np.float32), shard)
              for _ in range(2)]
    outs = fn(*dev_in)
    jax.block_until_ready(outs)
    t0 = time.perf_counter()
    outs_list = [fn(*dev_in) for _ in range(iters)]
    jax.block_until_ready(outs_list)
    return (time.perf_counter() - t0) / iters


def kernel(x0, x1, x2, x3, boxes):
    from concourse.bass_utils import run_bass_kernel_spmd
    in_maps = _host_prepare(x0, x1, x2, x3, boxes)
    if 'nc' not in _MODULE_CACHE:
        _MODULE_CACHE['nc'] = _build_module()
    nc = _MODULE_CACHE['nc']
    res = run_bass_kernel_spmd(nc, in_maps, list(range(8)), trace=TRACE)
    LAST_RESULT['res'] = res
    LAST_RESULT['in_maps'] = in_maps
    outs = [res.results[k]["out"] for k in range(8)]
    full = np.concatenate(outs, axis=0).astype(np.float32)   # [1024, 49, 256]
    return np.ascontiguousarray(
        full.transpose(0, 2, 1)).reshape(1024, C, POOLED, POOLED)


# revision 51
# speedup vs baseline: 3.8625x; 3.8404x over previous
"""Multi-level ROI Align (FPN pooler, 4 levels summed) on 8 Trainium2 cores.

Strategy: shard ROIs across cores (core k: batch k//4, 128 ROIs). All gather
indices and bilinear weights are computed on host from `boxes`; the device
kernel does the heavy lifting: HBM pixel gathers (dma_gather) + weighted
scatter-reduction into 7x7 bins via PSUM-accumulating matmuls.

Per ROI, per level:
  out[bin, c] = sum_j W[j, bin] * G[j, c]
where G rows are gathered 2-px vectors (C=256/px) and W is built on device as
fixed_pattern * per-partition scalar (L0/L1) or host-baked dense (L2/L3).

v6:
- bf16 features/weights/output (fp32 PSUM accumulation).
- gather calls are block-batched (the Q7 SWDGE has ~1us fixed cost/call) but
  each call stays <= 1024 descriptors (hard SWDGE ring-carveout cap,
  empirically 1024 ok / 1536 hangs).
- L0 is split into even-row / odd-row feature copies: every bilinear sample
  reads one even and one odd row, so each ROI contributes exactly 196
  2-px elems per parity (idx fits int16 at 1-px granularity), padded to 256
  for per-ROI chunk purity. 8 matmuls/ROI, no 3-px overfetch.
- L1: 2-px elems, 392 j's = 384 in block calls + 8 in one upfront tail call
  (tail chunks hold 16 ROIs; 16 static one-hot patterns, bins fixed).
- L2/L3: region pixels, 324/100 padded with idx=0 to 384/128.
- all 20+2 weight matrices of a ROI are built by 2 DVE tensor_tensor ops
  (pattern blocks * wcol columns broadcast via a 0-stride AP).
- output written [roi, 49, 256] bf16; host transposes + casts to fp32.
"""
import os
import sys
import numpy as np
import ml_dtypes

sys.path.insert(0, '/opt/trn_rl_repo')

BF16 = ml_dtypes.bfloat16
F8 = ml_dtypes.float8_e4m3
FP8_L23 = os.environ.get("K_FP8", "0") == "1"   # fp8 L2/L3: fails 2e-2 budget

POOLED = 7
SAMP = 2
NBIN = 49
C = 256
IMG = 800.0

NSAMP = 196         # samples per ROI (7x2 x 7x2)
L0P = 256           # padded per-parity list length (2 chunks)
REAL = 392          # L1 j's per ROI (2 rowsel x 196)
MAIN = 384          # L1 j's in block calls
TAIL = 8            # L1 j's in the shared tail call
TAIL_BINS = [45, 45, 46, 46, 47, 47, 48, 48]   # bins of L1 j 384..391

L0 = dict(H=200, W=200, scale=0.25)
L1 = dict(H=100, W=100, scale=0.125)
L2 = dict(H=50, W=50, scale=0.0625, NJ=384, REAL=324, NCH=3, WREG=18)
L3 = dict(H=25, W=25, scale=0.03125, NJ=128, REAL=100, NCH=1, WREG=10)

NROI_CORE = 128     # ROIs per core
BLK = 4             # ROIs per gather-call block
NBLK = NROI_CORE // BLK

# padded flat pixel counts of the feature buffers
F0P_ROWS = 20004    # per-parity f0 (100 rows x 200 px + 2-px overrun)
F1_ROWS = 10004     # covers 2-px elem overrun
F2_ROWS = 3400      # covers region overrun (y,x up to 66)
F3_ROWS = 900       # covers region overrun (y,x up to 33)

# const bf16 column layout (per partition): pre-tiled pattern blocks.
#   PATWM [14, 49]: k<8 -> PATP[:, (k%4)//2] (L0: par*4+c*2+s)
#                   k 8..13 -> PATM[:, (k-8)//2] (L1 main c*2+s)
#   PATWT [16, 2, 49]: variant r = roi%16, 2 copies of PATT_r (L1 tail)
PATWM_OFF = 0
PATWT_OFF = PATWM_OFF + 14 * NBIN
CST_COLS = PATWT_OFF + 16 * 2 * NBIN
# bf16 per-ROI scalar weight columns, 16 per roi:
#   0..7   L0 (par*4 + c*2 + s), 8..13 L1 main (c*2+s)  [14 "main" cols]
#   14..15 L1 tail (s)                                  [2 "tail" cols]
WPR = 16
WCOL_COLS = NROI_CORE * WPR

# idx int16 column layout (per partition)
IC0 = BLK * L0P // 16       # 64 cols per block per parity
IC1 = BLK * MAIN // 16      # 96
IC2 = BLK * L2['NJ'] // 16  # 96
IC3 = BLK * L3['NJ'] // 16  # 32
ICT = NROI_CORE * TAIL // 16    # 64 cols, L1 tail list
IDX0E_OFF = 0
IDX0O_OFF = IDX0E_OFF + NBLK * IC0
IDX1_OFF = IDX0O_OFF + NBLK * IC0
IDX2_OFF = IDX1_OFF + NBLK * IC1
IDX3_OFF = IDX2_OFF + NBLK * IC2
IDXT1_OFF = IDX3_OFF + NBLK * IC3
IDX_COLS = IDXT1_OFF + ICT

_MODULE_CACHE = {}
TRACE = False
LAST_RESULT = {}


def _sample_meta(boxes_b, H, W, scale):
    """Per-ROI sample geometry in fp32, matching reference op order.
    boxes_b: [N, 4] fp32. Returns dict of [N,7,2] arrays."""
    f = np.float32
    b = boxes_b.astype(np.float32)
    x1 = b[:, 0] * f(scale)
    y1 = b[:, 1] * f(scale)
    x2 = b[:, 2] * f(scale)
    y2 = b[:, 3] * f(scale)
    rw = np.maximum(x2 - x1, f(1.0))
    rh = np.maximum(y2 - y1, f(1.0))
    bw = rw / f(POOLED)
    bh = rh / f(POOLED)
    g = (np.arange(POOLED, dtype=np.float32)[:, None]
         + (np.arange(SAMP, dtype=np.float32)[None, :] + f(0.5)) / f(SAMP))
    y = y1[:, None, None] + g[None] * bh[:, None, None]   # [N,7,2]
    x = x1[:, None, None] + g[None] * bw[:, None, None]
    masky = ((y >= f(-1.0)) & (y <= f(H))).astype(np.float32)
    maskx = ((x >= f(-1.0)) & (x <= f(W))).astype(np.float32)
    yc = np.clip(y, f(0.0), f(H - 1))
    xc = np.clip(x, f(0.0), f(W - 1))
    yl = np.floor(yc).astype(np.int64)
    xl = np.floor(xc).astype(np.int64)
    yh = np.minimum(yl + 1, H - 1)
    xh = np.minimum(xl + 1, W - 1)
    ly = (yc - yl.astype(np.float32)).astype(np.float32)
    lx = (xc - xl.astype(np.float32)).astype(np.float32)
    hy = (f(1.0) - ly).astype(np.float32)
    hx = (f(1.0) - lx).astype(np.float32)
    return dict(yl=yl, yh=yh, xl=xl, xh=xh, ly=ly, lx=lx, hy=hy, hx=hx,
                masky=masky, maskx=maskx, x=x, y=y)


def _build_l0_parity(meta, lv):
    """L0 even/odd-row split. Per parity: 196 2-px elems in (py,sy,px,sx)
    order. Returns (idx_e, w_e, idx_o, w_o): idx [N,196] in parity-local px
    units, w [N,196,2]."""
    N = meta['yl'].shape[0]
    H, W = lv['H'], lv['W']
    sh = (N, 7, 2, 7, 2)
    yl = meta['yl']
    even = (yl % 2 == 0)
    # even-row: yl itself when even, else yl+1 (clamped: yl=H-1 odd -> weight
    # is ly=0 exactly, point at yl-1 harmlessly)
    ye = np.where(even, yl, np.where(yl == H - 1, yl - 1, yl + 1))
    yo = np.where(even, yl + 1, yl)
    wy_e = np.where(even, meta['hy'], meta['ly']) * meta['masky']
    wy_o = np.where(even, meta['ly'], meta['hy']) * meta['masky']

    def expand(yv, wyv):
        row = np.broadcast_to(yv[:, :, :, None, None], sh)
        wy = np.broadcast_to(wyv[:, :, :, None, None], sh).astype(np.float32)
        xl = np.broadcast_to(meta['xl'][:, None, None, :, :], sh)
        hx = np.broadcast_to(meta['hx'][:, None, None, :, :], sh).astype(np.float32)
        lx = np.broadcast_to(meta['lx'][:, None, None, :, :], sh).astype(np.float32)
        mx = np.broadcast_to(meta['maskx'][:, None, None, :, :], sh).astype(np.float32)
        idx = ((row >> 1) * W + xl).reshape(N, NSAMP)
        w = np.zeros((N, NSAMP, 2), np.float32)
        w[:, :, 0] = (wy * hx * mx * np.float32(0.25)).reshape(N, NSAMP)
        w[:, :, 1] = (wy * lx * mx * np.float32(0.25)).reshape(N, NSAMP)
        return idx, w

    idx_e, w_e = expand(ye, wy_e)
    idx_o, w_o = expand(yo, wy_o)
    return idx_e, w_e, idx_o, w_o


def _build_px2(meta, lv):
    """L1: 2-px elems, j=(rs,py,sy,px,sx). idx [N,392], w [N,392,2]."""
    N = meta['yl'].shape[0]
    W = lv['W']
    sh = (N, 2, 7, 2, 7, 2)
    rows = np.stack([meta['yl'], meta['yh']], axis=1)          # [N,2,7,2]
    wys = np.stack([meta['hy'], meta['ly']], axis=1)
    m = (meta['masky'][:, :, :, None, None] * meta['maskx'][:, None, None, :, :])
    row = np.broadcast_to(rows[:, :, :, :, None, None], sh)
    wy = np.broadcast_to(wys[:, :, :, :, None, None], sh).astype(np.float32)
    xl = np.broadcast_to(meta['xl'][:, None, None, None, :, :], sh)
    hx = np.broadcast_to(meta['hx'][:, None, None, None, :, :], sh).astype(np.float32)
    lx = np.broadcast_to(meta['lx'][:, None, None, None, :, :], sh).astype(np.float32)
    mm = np.broadcast_to(m[:, None], sh).astype(np.float32)
    flat = (row * W + xl).reshape(N, REAL)
    w = np.zeros((N, REAL, 2), np.float32)
    w[:, :, 0] = (wy * hx * mm * np.float32(0.25)).reshape(N, REAL)
    w[:, :, 1] = (wy * lx * mm * np.float32(0.25)).reshape(N, REAL)
    return flat, w


def _build_reg(meta, lv):
    """L2/L3: bounding-region pixels + separable host-baked weights.
    Returns idx [N, NJ] int64 (pad idx=0), lhsT [N, NJ, 49] fp32."""
    N = meta['yl'].shape[0]
    H, W, WREG = lv['H'], lv['W'], lv['WREG']
    NJ, RL = lv['NJ'], lv['REAL']
    f = np.float32
    y_base = np.floor(np.clip(meta['y'].reshape(N, -1).min(1), 0.0, H - 1)).astype(np.int64)
    x_base = np.floor(np.clip(meta['x'].reshape(N, -1).min(1), 0.0, W - 1)).astype(np.int64)
    WY = np.zeros((N, WREG, POOLED), np.float32)
    WX = np.zeros((N, WREG, POOLED), np.float32)
    ridx = np.arange(N)[:, None, None]
    pidx = np.broadcast_to(np.arange(POOLED)[None, :, None], (N, POOLED, SAMP))
    np.add.at(WY, (ridx, meta['yl'] - y_base[:, None, None], pidx),
              (f(0.5) * meta['hy'] * meta['masky']).astype(np.float32))
    np.add.at(WY, (ridx, meta['yh'] - y_base[:, None, None], pidx),
              (f(0.5) * meta['ly'] * meta['masky']).astype(np.float32))
    np.add.at(WX, (ridx, meta['xl'] - x_base[:, None, None], pidx),
              (f(0.5) * meta['hx'] * meta['maskx']).astype(np.float32))
    np.add.at(WX, (ridx, meta['xh'] - x_base[:, None, None], pidx),
              (f(0.5) * meta['lx'] * meta['maskx']).astype(np.float32))
    lhsT = np.einsum('nap,nbq->nabpq', WY, WX).reshape(N, RL, NBIN)
    dy = np.arange(WREG)
    idx = ((y_base[:, None, None] + dy[None, :, None]) * W
           + x_base[:, None, None] + dy[None, None, :]).reshape(N, RL)
    idx_full = np.zeros((N, NJ), np.int64)
    lhsT_full = np.zeros((N, NJ, NBIN), np.float32)
    idx_full[:, :RL] = idx
    lhsT_full[:, :RL] = lhsT
    return idx_full, lhsT_full


def _pack_idx(jlists):
    """Pack idx list [..., NJ_total] -> [..., 128, NJ_total//16] int16
    wrapped in 16 partitions, replicated 8x."""
    jl = np.asarray(jlists)
    n = jl.shape[-1]
    arr = jl.reshape(*jl.shape[:-1], n // 16, 16)   # [..., col, p]
    arr = np.swapaxes(arr, -1, -2)                  # [..., p(16), col]
    arr = np.broadcast_to(arr[..., None, :, :],
                          (*jl.shape[:-1], 8, 16, n // 16))
    return arr.reshape(*jl.shape[:-1], 128, n // 16).astype(np.int16)


def _patterns():
    """PATP [128, 2, 49]: L0 parity-sample pattern (k=(py,sy,px,sx) order,
    196 real). PATM [128, 3, 49]: L1 main (j 0..383 of the 392-order).
    PATT [128, 16, 49]: L1 tail variants."""
    k = np.arange(2 * 128)
    px = (k // 2) % 7
    py = k // 28
    patp = np.zeros((2 * 128, NBIN), np.float32)
    v = k < NSAMP
    patp[np.arange(2 * 128)[v], (py * 7 + px)[v]] = 1.0
    patp = patp.reshape(2, 128, NBIN).transpose(1, 0, 2)

    j = np.arange(MAIN)
    px = (j // 2) % 7
    py = (j // 28) % 7
    patm = np.zeros((MAIN, NBIN), np.float32)
    patm[np.arange(MAIN), py * 7 + px] = 1.0
    patm = patm.reshape(3, 128, NBIN).transpose(1, 0, 2)

    patt = np.zeros((128, 16, NBIN), np.float32)
    for r in range(16):
        for kk in range(TAIL):
            patt[r * TAIL + kk, r, TAIL_BINS[kk]] = 1.0
    return patp, patm, patt


def _host_prepare(x0, x1, x2, x3, boxes):
    """Build all per-core input tensors. Returns list of 8 dicts."""
    B = boxes.shape[0]
    # f0 split by row parity
    f0b = np.transpose(np.asarray(x0, np.float32), (0, 2, 3, 1))   # [B,H,W,C]
    f0e = np.zeros((B, F0P_ROWS, C), BF16)
    f0o = np.zeros((B, F0P_ROWS, C), BF16)
    f0e[:, :100 * 200] = f0b[:, 0::2].reshape(B, -1, C).astype(BF16)
    f0o[:, :100 * 200] = f0b[:, 1::2].reshape(B, -1, C).astype(BF16)
    feats = [f0e, f0o]
    dt23 = F8 if FP8_L23 else BF16
    for arr, lv, rows, dt in ((x1, L1, F1_ROWS, BF16), (x2, L2, F2_ROWS, dt23),
                              (x3, L3, F3_ROWS, dt23)):
        f = np.zeros((B, rows, C), dt)
        hw = lv['H'] * lv['W']
        f[:, :hw] = np.ascontiguousarray(
            np.transpose(np.asarray(arr, np.float32), (0, 2, 3, 1))
        ).reshape(B, hw, C).astype(dt)
        feats.append(f)

    per_batch = []
    for b in range(B):
        bb = np.asarray(boxes[b], np.float32)
        m0 = _sample_meta(bb, L0['H'], L0['W'], L0['scale'])
        m1 = _sample_meta(bb, L1['H'], L1['W'], L1['scale'])
        m2 = _sample_meta(bb, L2['H'], L2['W'], L2['scale'])
        m3 = _sample_meta(bb, L3['H'], L3['W'], L3['scale'])
        i0e, w0e, i0o, w0o = _build_l0_parity(m0, L0)
        idx1, w1 = _build_px2(m1, L1)
        idx2, lt2 = _build_reg(m2, L2)
        idx3, lt3 = _build_reg(m3, L3)
        per_batch.append((i0e, w0e, i0o, w0o, idx1, w1, idx2, lt2, idx3, lt3))

    patp, patm, patt = _patterns()

    in_maps = []
    for k in range(8):
        b = k // 4
        s = (k % 4) * NROI_CORE
        i0e, w0e, i0o, w0o, idx1, w1, idx2, lt2, idx3, lt3 = per_batch[b]
        sl = slice(s, s + NROI_CORE)

        cst = np.zeros((128, CST_COLS), BF16)
        patwm = np.concatenate(
            [patp[:, [(k_ % 4) // 2 for k_ in range(8)], :],
             patm[:, [k_ // 2 for k_ in range(6)], :]], axis=1)  # [128,14,49]
        cst[:, PATWM_OFF:PATWM_OFF + 14 * NBIN] = patwm.reshape(128, -1).astype(BF16)
        patwt = np.broadcast_to(patt[:, :, None, :], (128, 16, 2, NBIN))
        cst[:, PATWT_OFF:PATWT_OFF + 16 * 2 * NBIN] = patwt.reshape(128, -1).astype(BF16)

        # wcol [128, roi*16 + k]
        wcol = np.zeros((128, WCOL_COLS), np.float32)
        cols = np.arange(NROI_CORE) * WPR
        # L0: pad [N,196,2] -> [N,256,2]; col par*4 + c*2 + s
        for par, warr in ((0, w0e[sl]), (1, w0o[sl])):
            wp_ = np.zeros((NROI_CORE, L0P, 2), np.float32)
            wp_[:, :NSAMP] = warr
            wp_ = wp_.reshape(NROI_CORE, 2, 128, 2)          # [roi, c, p, s]
            for c in range(2):
                for s2 in range(2):
                    wcol[:, cols + par * 4 + c * 2 + s2] = wp_[:, c, :, s2].T
        # L1 main
        w1c = w1[sl]
        wm1 = w1c[:, :MAIN].reshape(NROI_CORE, 3, 128, 2)
        for c in range(3):
            for s2 in range(2):
                wcol[:, cols + 8 + c * 2 + s2] = wm1[:, c, :, s2].T
        # L1 tail: value at partition (roi%16)*8 + kk
        prt = (np.arange(NROI_CORE) % 16)[:, None] * TAIL + np.arange(TAIL)[None, :]
        for s2 in range(2):
            wt_ = np.zeros((NROI_CORE, 128), np.float32)
            np.put_along_axis(wt_, prt, w1c[:, MAIN:, s2], axis=1)
            wcol[:, cols + 14 + s2] = wt_.T
        wcol = wcol.astype(BF16)

        # idx lists
        def padl(a, n):
            out = np.zeros((a.shape[0], n), np.int64)
            out[:, :a.shape[1]] = a
            return out

        idxs = np.zeros((128, IDX_COLS), np.int16)
        idxs[:, IDX0E_OFF:IDX0E_OFF + NBLK * IC0] = _pack_idx(
            padl(i0e[sl], L0P).reshape(NBLK, BLK * L0P)
        ).transpose(1, 0, 2).reshape(128, -1)
        idxs[:, IDX0O_OFF:IDX0O_OFF + NBLK * IC0] = _pack_idx(
            padl(i0o[sl], L0P).reshape(NBLK, BLK * L0P)
        ).transpose(1, 0, 2).reshape(128, -1)
        idxs[:, IDX1_OFF:IDX1_OFF + NBLK * IC1] = _pack_idx(
            idx1[sl][:, :MAIN].reshape(NBLK, BLK * MAIN)
        ).transpose(1, 0, 2).reshape(128, -1)
        idxs[:, IDX2_OFF:IDX2_OFF + NBLK * IC2] = _pack_idx(
            idx2[sl].reshape(NBLK, BLK * L2['NJ'])
        ).transpose(1, 0, 2).reshape(128, -1)
        idxs[:, IDX3_OFF:IDX3_OFF + NBLK * IC3] = _pack_idx(
            idx3[sl].reshape(NBLK, BLK * L3['NJ'])
        ).transpose(1, 0, 2).reshape(128, -1)
        idxs[:, IDXT1_OFF:IDXT1_OFF + ICT] = _pack_idx(
            idx1[sl][:, MAIN:].reshape(NROI_CORE * TAIL))

        # lhsT k-major: lt2 [roi, 3*128, 49] -> [roi, 128, 3, 49];
        # lt3 [roi, 128, 49]; combined [roi, 128, 4, 49] bf16
        lt2k = lt2[sl].reshape(NROI_CORE, L2['NCH'], 128, NBIN).transpose(0, 2, 1, 3)
        lt3k = lt3[sl].reshape(NROI_CORE, 1, 128, NBIN).transpose(0, 2, 1, 3)
        lt = np.ascontiguousarray(
            np.concatenate([lt2k, lt3k], axis=2)).astype(dt23)

        in_maps.append({
            "f0e": feats[0][b], "f0o": feats[1][b], "f1": feats[2][b],
            "f2": feats[3][b], "f3": feats[4][b],
            "cst": cst, "wcol": wcol, "idxs": idxs, "lt": lt,
        })
    return in_maps


def _build_module():
    from concourse import bacc, tile
    from concourse.bass import mybir
    import concourse.bass as bass_mod

    F32 = mybir.dt.float32
    BF = mybir.dt.bfloat16
    D23 = mybir.dt.float8e4 if FP8_L23 else BF
    I16 = mybir.dt.int16
    AP = bass_mod.AP

    nc = bacc.Bacc(None, target_bir_lowering=False)
    f0e = nc.dram_tensor("f0e", [F0P_ROWS, C], BF, kind="ExternalInput")
    f0o = nc.dram_tensor("f0o", [F0P_ROWS, C], BF, kind="ExternalInput")
    f1 = nc.dram_tensor("f1", [F1_ROWS, C], BF, kind="ExternalInput")
    f2 = nc.dram_tensor("f2", [F2_ROWS, C], D23, kind="ExternalInput")
    f3 = nc.dram_tensor("f3", [F3_ROWS, C], D23, kind="ExternalInput")
    cst = nc.dram_tensor("cst", [128, CST_COLS], BF, kind="ExternalInput")
    wcol = nc.dram_tensor("wcol", [128, WCOL_COLS], BF, kind="ExternalInput")
    idxs = nc.dram_tensor("idxs", [128, IDX_COLS], I16, kind="ExternalInput")
    lt = nc.dram_tensor("lt", [NROI_CORE, 128, 4, NBIN], D23, kind="ExternalInput")
    out = nc.dram_tensor("out", [NROI_CORE, NBIN, C], BF, kind="ExternalOutput")

    # 2-px-elem views at 1-px stride
    f0eV = AP(f0e, 0, [[C, F0P_ROWS - 2], [1, 2 * C]])
    f0oV = AP(f0o, 0, [[C, F0P_ROWS - 2], [1, 2 * C]])
    f1v = AP(f1, 0, [[C, F1_ROWS - 2], [1, 2 * C]])

    with tile.TileContext(nc) as tc:
        with (
            tc.tile_pool(name="const", bufs=1) as constp,
            tc.tile_pool(name="g0ep", bufs=2) as g0ep,
            tc.tile_pool(name="g0op", bufs=2) as g0op,
            tc.tile_pool(name="g1p", bufs=2) as g1p,
            tc.tile_pool(name="g2p", bufs=2) as g2p,
            tc.tile_pool(name="g3p", bufs=2) as g3p,
            tc.tile_pool(name="ltp", bufs=3) as ltp,
            tc.tile_pool(name="wp", bufs=4) as wp,
            tc.tile_pool(name="accp", bufs=4, space="PSUM") as accp,
            tc.tile_pool(name="evp", bufs=3) as evp,
        ):
            cst_t = constp.tile([128, CST_COLS], BF)
            nc.sync.dma_start(cst_t[:], cst[:])
            wcol_t = constp.tile([128, WCOL_COLS], BF, tag="wcol")
            nc.sync.dma_start(wcol_t[:], wcol[:])
            idx_t = constp.tile([128, IDX_COLS], I16)
            nc.sync.dma_start(idx_t[:], idxs[:])

            # upfront L1 tail gather: all 128 ROIs' last-8 j's
            g1T = constp.tile([128, NROI_CORE // 16, 2 * C], BF, tag="g1T")
            nc.gpsimd.dma_gather(
                g1T[:], f1v, idx_t[:, IDXT1_OFF:IDXT1_OFF + ICT],
                NROI_CORE * TAIL, NROI_CORE * TAIL, 2 * C, elem_step=C)

            for blk in range(NBLK):
                # L0 parity gathers: BLK*256 = 1024 descs each (at the cap)
                gt0e = g0ep.tile([128, BLK * 2, 2 * C], BF, tag="g0e")
                io = IDX0E_OFF + blk * IC0
                nc.gpsimd.dma_gather(
                    gt0e[:], f0eV, idx_t[:, io:io + IC0],
                    BLK * L0P, BLK * L0P, 2 * C, elem_step=C)
                gt0o = g0op.tile([128, BLK * 2, 2 * C], BF, tag="g0o")
                io = IDX0O_OFF + blk * IC0
                nc.gpsimd.dma_gather(
                    gt0o[:], f0oV, idx_t[:, io:io + IC0],
                    BLK * L0P, BLK * L0P, 2 * C, elem_step=C)
                # L1/L2: 2-ROI halves keep calls at 768 descs
                gt1 = g1p.tile([128, BLK * 3, 2 * C], BF, tag="g1")
                io = IDX1_OFF + blk * IC1
                for h in range(2):
                    nc.gpsimd.dma_gather(
                        gt1[:, h * 6:(h + 1) * 6, :], f1v,
                        idx_t[:, io + h * 48:io + (h + 1) * 48],
                        2 * MAIN, 2 * MAIN, 2 * C, elem_step=C)
                gt2 = g2p.tile([128, BLK * 3, C], D23, tag="g2")
                io = IDX2_OFF + blk * IC2
                for h in range(2):
                    nc.gpsimd.dma_gather(
                        gt2[:, h * 6:(h + 1) * 6, :], f2[:],
                        idx_t[:, io + h * 48:io + (h + 1) * 48],
                        2 * L2['NJ'], 2 * L2['NJ'], C, elem_step=C)
                gt3 = g3p.tile([128, BLK, C], D23, tag="g3")
                io = IDX3_OFF + blk * IC3
                nc.gpsimd.dma_gather(
                    gt3[:], f3[:], idx_t[:, io:io + IC3],
                    BLK * L3['NJ'], BLK * L3['NJ'], C, elem_step=C)

                for bri in range(BLK):
                    roi = blk * BLK + bri
                    tr = roi % 16            # tail pattern index
                    tch = roi // 16          # tail chunk
                    lt_t = ltp.tile([128, 4, NBIN], D23, tag="lt")
                    nc.sync.dma_start(lt_t[:], lt[roi])

                    acc = accp.tile([NBIN, C], F32)
                    n_mm = 8 + 8 + 3 + 1
                    mi = 0
                    wb = roi * WPR

                    # build all weight mats in 2 DVE ops: pattern blocks *
                    # wcol columns broadcast (0-stride) across the 49 bins
                    wm = wp.tile([128, 14, NBIN], BF, tag="wm")
                    wsl = wcol_t[:, wb:wb + 14]
                    nc.vector.tensor_mul(
                        wm[:],
                        cst_t[:, PATWM_OFF:PATWM_OFF + 14 * NBIN].rearrange(
                            "p (k b) -> p k b", b=NBIN),
                        AP(wsl.tensor, wsl.offset, [*wsl.ap, [0, NBIN]]))
                    wt = wp.tile([128, 2, NBIN], BF, tag="wt")
                    tsl = wcol_t[:, wb + 14:wb + 16]
                    to = PATWT_OFF + tr * 2 * NBIN
                    nc.vector.tensor_mul(
                        wt[:],
                        cst_t[:, to:to + 2 * NBIN].rearrange(
                            "p (k b) -> p k b", b=NBIN),
                        AP(tsl.tensor, tsl.offset, [*tsl.ap, [0, NBIN]]))

                    # L0: 2 parities x 2 chunks x 2 slots
                    for par, gt0 in ((0, gt0e), (1, gt0o)):
                        for c in range(2):
                            for s2 in range(2):
                                nc.tensor.matmul(
                                    acc[:], wm[:, par * 4 + c * 2 + s2, :],
                                    gt0[:, bri * 2 + c, s2 * C:(s2 + 1) * C],
                                    start=(mi == 0), stop=(mi == n_mm - 1))
                                mi += 1
                    # L1 main: 3 chunks x 2 slots
                    for c in range(3):
                        for s2 in range(2):
                            nc.tensor.matmul(
                                acc[:], wm[:, 8 + c * 2 + s2, :],
                                gt1[:, bri * 3 + c, s2 * C:(s2 + 1) * C],
                                start=(mi == 0), stop=(mi == n_mm - 1))
                            mi += 1
                    # L1 tail: 2 slots
                    for s2 in range(2):
                        nc.tensor.matmul(
                            acc[:], wt[:, s2, :], g1T[:, tch, s2 * C:(s2 + 1) * C],
                            start=(mi == 0), stop=(mi == n_mm - 1))
                        mi += 1
                    # L2: 3 chunks, host-baked lhsT
                    for c in range(3):
                        nc.tensor.matmul(
                            acc[:], lt_t[:, c, :], gt2[:, bri * 3 + c, :],
                            start=(mi == 0), stop=(mi == n_mm - 1))
                        mi += 1
                    # L3: 1 chunk
                    nc.tensor.matmul(
                        acc[:], lt_t[:, 3, :], gt3[:, bri, :],
                        start=(mi == 0), stop=(mi == n_mm - 1))
                    mi += 1

                    ev = evp.tile([NBIN, C], BF, tag="ev")
                    nc.scalar.copy(ev[:], acc[:])
                    nc.sync.dma_start(out[roi], ev[:])
    nc.finalize()
    return nc


def bench(iters=12):
    """Device-time estimate: repeat-run the compiled module with pre-staged
    device inputs (no donation). Returns (per_iter_times_s, chained_avg_s)."""
    import time
    import jax
    from jax.experimental.shard_map import shard_map
    from jax.sharding import Mesh, NamedSharding, PartitionSpec
    from concourse import bass2jax
    from concourse.bass import mybir

    nc = _MODULE_CACHE['nc']
    in_maps = LAST_RESULT['in_maps']
    bass2jax.install_neuronx_cc_hook()
    pname = nc.partition_id_tensor.name if nc.partition_id_tensor else None
    in_names, out_names, out_avals, zero_outs = [], [], [], []
    for alloc in nc.m.functions[0].allocations:
        if not isinstance(alloc, mybir.MemoryLocationSet):
            continue
        name = alloc.memorylocations[0].name
        if alloc.kind == "ExternalInput":
            if name != pname:
                in_names.append(name)
        elif alloc.kind == "ExternalOutput":
            shape = tuple(alloc.tensor_shape)
            dtype = mybir.dt.np(alloc.dtype)
            out_names.append(name)
            out_avals.append(jax.core.ShapedArray(shape, dtype))
            zero_outs.append(np.zeros(shape, dtype))
    n_params = len(in_names)
    in_names_all = in_names + out_names + ([pname] if pname else [])

    def _body(*args):
        operands = list(args)
        if pname is not None:
            operands.append(bass2jax.partition_id_tensor())
        return tuple(bass2jax._bass_exec_p.bind(
            *operands,
            out_avals=tuple(out_avals),
            in_names=tuple(in_names_all),
            out_names=tuple(out_names),
            lowering_input_output_aliases=(),
            sim_require_finite=True,
            sim_require_nnan=True,
            nc=nc,
        ))

    n_cores = 8
    devices = jax.devices()[:n_cores]
    mesh = Mesh(np.asarray(devices), ("core",))
    nio = n_params + len(out_names)
    fn = jax.jit(
        shard_map(_body, mesh=mesh, in_specs=(PartitionSpec("core"),) * nio,
                  out_specs=(PartitionSpec("core"),) * len(out_names),
                  check_rep=False),
        keep_unused=True)
    per_core = [[np.asarray(m[name]) for name in in_names] for m in in_maps]
    concat_in = [np.concatenate([per_core[c][i] for c in range(n_cores)], axis=0)
                 for i in range(n_params)]
    concat_zeros = [np.zeros((n_cores * z.shape[0], *z.shape[1:]), z.dtype)
                    for z in zero_outs]
    shard = NamedSharding(mesh, PartitionSpec("core"))
    dev_in = [jax.device_put(a, shard) for a in concat_in + concat_zeros]
    outs = fn(*dev_in)
    jax.block_until_ready(outs)
    times = []
    for _ in range(iters):
        t0 = time.perf_counter()
        outs = fn(*dev_in)
        jax.block_until_ready(outs)
        times.append(time.perf_counter() - t0)
    t0 = time.perf_counter()
    outs_list = [fn(*dev_in) for _ in range(iters)]
    jax.block_until_ready(outs_list)
    chained = (time.perf_counter() - t0) / iters
    return times, chained


def bench_floor(iters=12):
    """Chained-dispatch floor: same bench loop on a trivial copy kernel.
    Subtracting this from bench()'s chained average isolates device time."""
    import time
    import jax
    from jax.experimental.shard_map import shard_map
    from jax.sharding import Mesh, NamedSharding, PartitionSpec
    from concourse import bacc, tile, bass2jax
    from concourse.bass import mybir

    if 'tiny' not in _MODULE_CACHE:
        F32 = mybir.dt.float32
        nct = bacc.Bacc(None, target_bir_lowering=False)
        xin = nct.dram_tensor("xin", [128, 128], F32, kind="ExternalInput")
        xout = nct.dram_tensor("xout", [128, 128], F32, kind="ExternalOutput")
        with tile.TileContext(nct) as tc:
            with tc.tile_pool(name="p", bufs=1) as p:
                t = p.tile([128, 128], F32)
                nct.sync.dma_start(t[:], xin[:])
                nct.sync.dma_start(xout[:], t[:])
        nct.finalize()
        _MODULE_CACHE['tiny'] = nct
    nct = _MODULE_CACHE['tiny']
    bass2jax.install_neuronx_cc_hook()
    pname = nct.partition_id_tensor.name if nct.partition_id_tensor else None
    out_avals = [jax.core.ShapedArray((128, 128), np.float32)]

    def _body(*args):
        operands = list(args)
        if pname is not None:
            operands.append(bass2jax.partition_id_tensor())
        return tuple(bass2jax._bass_exec_p.bind(
            *operands,
            out_avals=tuple(out_avals),
            in_names=("xin", "xout") + ((pname,) if pname else ()),
            out_names=("xout",),
            lowering_input_output_aliases=(),
            sim_require_finite=True,
            sim_require_nnan=True,
            nc=nct,
        ))

    n_cores = 8
    devices = jax.devices()[:n_cores]
    mesh = Mesh(np.asarray(devices), ("core",))
    fn = jax.jit(
        shard_map(_body, mesh=mesh, in_specs=(PartitionSpec("core"),) * 2,
                  out_specs=(PartitionSpec("core"),), check_rep=False),
        keep_unused=True)
    shard = NamedSharding(mesh, PartitionSpec("core"))
    dev_in = [jax.device_put(np.zeros((n_cores * 128, 128), np.float32), shard)
              for _ in range(2)]
    outs = fn(*dev_in)
    jax.block_until_ready(outs)
    t0 = time.perf_counter()
    outs_list = [fn(*dev_in) for _ in range(iters)]
    jax.block_until_ready(outs_list)
    return (time.perf_counter() - t0) / iters


def kernel(x0, x1, x2, x3, boxes):
    from concourse.bass_utils import run_bass_kernel_spmd
    in_maps = _host_prepare(x0, x1, x2, x3, boxes)
    if 'nc' not in _MODULE_CACHE:
        _MODULE_CACHE['nc'] = _build_module()
    nc = _MODULE_CACHE['nc']
    res = run_bass_kernel_spmd(nc, in_maps, list(range(8)), trace=TRACE)
    LAST_RESULT['res'] = res
    LAST_RESULT['in_maps'] = in_maps
    outs = [res.results[k]["out"] for k in range(8)]
    full = np.concatenate(outs, axis=0).astype(np.float32)   # [1024, 49, 256]
    return np.ascontiguousarray(
        full.transpose(0, 2, 1)).reshape(1024, C, POOLED, POOLED)


# revision 52
# speedup vs baseline: 85.6283x; 22.1689x over previous
"""Multi-level ROI Align (FPN pooler, 4 levels summed) on 8 Trainium2 cores.

Strategy: shard ROIs across cores (core k: batch k//4, 128 ROIs). All gather
indices and bilinear weights are computed on host from `boxes`; the device
kernel does the heavy lifting: HBM pixel gathers (dma_gather) + weighted
scatter-reduction into 7x7 bins via PSUM-accumulating matmuls.

Per ROI, per level:
  out[bin, c] = sum_j W[j, bin] * G[j, c]
where G rows are gathered 2-px vectors (C=256/px) and W is built on device as
fixed_pattern * per-partition scalar (L0/L1) or host-baked dense (L2/L3).

v6:
- bf16 features/weights/output (fp32 PSUM accumulation).
- gather calls are block-batched (the Q7 SWDGE has ~1us fixed cost/call) but
  each call stays <= 1024 descriptors (hard SWDGE ring-carveout cap,
  empirically 1024 ok / 1536 hangs).
- L0 is split into even-row / odd-row feature copies: every bilinear sample
  reads one even and one odd row, so each ROI contributes exactly 196
  2-px elems per parity (idx fits int16 at 1-px granularity), padded to 256
  for per-ROI chunk purity. 8 matmuls/ROI, no 3-px overfetch.
- L1: 2-px elems, 392 j's = 384 in block calls + 8 in one upfront tail call
  (tail chunks hold 16 ROIs; 16 static one-hot patterns, bins fixed).
- L2/L3: region pixels, 324/100 padded with idx=0 to 384/128.
- all 20+2 weight matrices of a ROI are built by 2 DVE tensor_tensor ops
  (pattern blocks * wcol columns broadcast via a 0-stride AP).
- output written [roi, 49, 256] bf16; host transposes + casts to fp32.
"""
import os
import sys
import numpy as np
import ml_dtypes

sys.path.insert(0, '/opt/trn_rl_repo')

BF16 = ml_dtypes.bfloat16
F8 = ml_dtypes.float8_e4m3
FP8_L23 = os.environ.get("K_FP8", "0") == "1"   # fp8 L2/L3: fails 2e-2 budget

POOLED = 7
SAMP = 2
NBIN = 49
C = 256
IMG = 800.0

NSAMP = 196         # samples per ROI (7x2 x 7x2)
L0P = 256           # padded per-parity list length (2 chunks)
REAL = 392          # L1 j's per ROI (2 rowsel x 196)
MAIN = 384          # L1 j's in block calls
TAIL = 8            # L1 j's in the shared tail call
TAIL_BINS = [45, 45, 46, 46, 47, 47, 48, 48]   # bins of L1 j 384..391

L0 = dict(H=200, W=200, scale=0.25)
L1 = dict(H=100, W=100, scale=0.125)
L2 = dict(H=50, W=50, scale=0.0625, NJ=384, REAL=324, NCH=3, WREG=18)
L3 = dict(H=25, W=25, scale=0.03125, NJ=128, REAL=100, NCH=1, WREG=10)

NROI_CORE = 128     # ROIs per core
BLK = 4             # ROIs per gather-call block
NBLK = NROI_CORE // BLK

# padded flat pixel counts of the feature buffers
F0P_ROWS = 20004    # per-parity f0 (100 rows x 200 px + 2-px overrun)
F1_ROWS = 10004     # covers 2-px elem overrun
F2_ROWS = 3400      # covers region overrun (y,x up to 66)
F3_ROWS = 900       # covers region overrun (y,x up to 33)

# const bf16 column layout (per partition): pre-tiled pattern blocks.
#   PATWM [14, 49]: k<8 -> PATP[:, (k%4)//2] (L0: par*4+c*2+s)
#                   k 8..13 -> PATM[:, (k-8)//2] (L1 main c*2+s)
#   PATWT [16, 2, 49]: variant r = roi%16, 2 copies of PATT_r (L1 tail)
PATWM_OFF = 0
PATWT_OFF = PATWM_OFF + 14 * NBIN
CST_COLS = PATWT_OFF + 16 * 2 * NBIN
# bf16 per-ROI scalar weight columns, 16 per roi:
#   0..7   L0 (par*4 + c*2 + s), 8..13 L1 main (c*2+s)  [14 "main" cols]
#   14..15 L1 tail (s)                                  [2 "tail" cols]
WPR = 16
WCOL_COLS = NROI_CORE * WPR

# idx int16 column layout (per partition)
IC0 = BLK * L0P // 16       # 64 cols per block per parity
IC1 = BLK * MAIN // 16      # 96
IC2 = BLK * L2['NJ'] // 16  # 96
IC3 = BLK * L3['NJ'] // 16  # 32
ICT = NROI_CORE * TAIL // 16    # 64 cols, L1 tail list
IDX0E_OFF = 0
IDX0O_OFF = IDX0E_OFF + NBLK * IC0
IDX1_OFF = IDX0O_OFF + NBLK * IC0
IDX2_OFF = IDX1_OFF + NBLK * IC1
IDX3_OFF = IDX2_OFF + NBLK * IC2
IDXT1_OFF = IDX3_OFF + NBLK * IC3
IDX_COLS = IDXT1_OFF + ICT

_MODULE_CACHE = {}
TRACE = False
LAST_RESULT = {}


def _sample_meta(boxes_b, H, W, scale):
    """Per-ROI sample geometry in fp32, matching reference op order.
    boxes_b: [N, 4] fp32. Returns dict of [N,7,2] arrays."""
    f = np.float32
    b = boxes_b.astype(np.float32)
    x1 = b[:, 0] * f(scale)
    y1 = b[:, 1] * f(scale)
    x2 = b[:, 2] * f(scale)
    y2 = b[:, 3] * f(scale)
    rw = np.maximum(x2 - x1, f(1.0))
    rh = np.maximum(y2 - y1, f(1.0))
    bw = rw / f(POOLED)
    bh = rh / f(POOLED)
    g = (np.arange(POOLED, dtype=np.float32)[:, None]
         + (np.arange(SAMP, dtype=np.float32)[None, :] + f(0.5)) / f(SAMP))
    y = y1[:, None, None] + g[None] * bh[:, None, None]   # [N,7,2]
    x = x1[:, None, None] + g[None] * bw[:, None, None]
    masky = ((y >= f(-1.0)) & (y <= f(H))).astype(np.float32)
    maskx = ((x >= f(-1.0)) & (x <= f(W))).astype(np.float32)
    yc = np.clip(y, f(0.0), f(H - 1))
    xc = np.clip(x, f(0.0), f(W - 1))
    yl = np.floor(yc).astype(np.int64)
    xl = np.floor(xc).astype(np.int64)
    yh = np.minimum(yl + 1, H - 1)
    xh = np.minimum(xl + 1, W - 1)
    ly = (yc - yl.astype(np.float32)).astype(np.float32)
    lx = (xc - xl.astype(np.float32)).astype(np.float32)
    hy = (f(1.0) - ly).astype(np.float32)
    hx = (f(1.0) - lx).astype(np.float32)
    return dict(yl=yl, yh=yh, xl=xl, xh=xh, ly=ly, lx=lx, hy=hy, hx=hx,
                masky=masky, maskx=maskx, x=x, y=y)


def _build_l0_parity(meta, lv):
    """L0 even/odd-row split. Per parity: 196 2-px elems in (py,sy,px,sx)
    order. Returns (idx_e, w_e, idx_o, w_o): idx [N,196] in parity-local px
    units, w [N,196,2]."""
    N = meta['yl'].shape[0]
    H, W = lv['H'], lv['W']
    sh = (N, 7, 2, 7, 2)
    yl = meta['yl']
    even = (yl % 2 == 0)
    # even-row: yl itself when even, else yl+1 (clamped: yl=H-1 odd -> weight
    # is ly=0 exactly, point at yl-1 harmlessly)
    ye = np.where(even, yl, np.where(yl == H - 1, yl - 1, yl + 1))
    yo = np.where(even, yl + 1, yl)
    wy_e = np.where(even, meta['hy'], meta['ly']) * meta['masky']
    wy_o = np.where(even, meta['ly'], meta['hy']) * meta['masky']

    def expand(yv, wyv):
        row = np.broadcast_to(yv[:, :, :, None, None], sh)
        wy = np.broadcast_to(wyv[:, :, :, None, None], sh).astype(np.float32)
        xl = np.broadcast_to(meta['xl'][:, None, None, :, :], sh)
        hx = np.broadcast_to(meta['hx'][:, None, None, :, :], sh).astype(np.float32)
        lx = np.broadcast_to(meta['lx'][:, None, None, :, :], sh).astype(np.float32)
        mx = np.broadcast_to(meta['maskx'][:, None, None, :, :], sh).astype(np.float32)
        idx = ((row >> 1) * W + xl).reshape(N, NSAMP)
        w = np.zeros((N, NSAMP, 2), np.float32)
        w[:, :, 0] = (wy * hx * mx * np.float32(0.25)).reshape(N, NSAMP)
        w[:, :, 1] = (wy * lx * mx * np.float32(0.25)).reshape(N, NSAMP)
        return idx, w

    idx_e, w_e = expand(ye, wy_e)
    idx_o, w_o = expand(yo, wy_o)
    return idx_e, w_e, idx_o, w_o


def _build_px2(meta, lv):
    """L1: 2-px elems, j=(rs,py,sy,px,sx). idx [N,392], w [N,392,2]."""
    N = meta['yl'].shape[0]
    W = lv['W']
    sh = (N, 2, 7, 2, 7, 2)
    rows = np.stack([meta['yl'], meta['yh']], axis=1)          # [N,2,7,2]
    wys = np.stack([meta['hy'], meta['ly']], axis=1)
    m = (meta['masky'][:, :, :, None, None] * meta['maskx'][:, None, None, :, :])
    row = np.broadcast_to(rows[:, :, :, :, None, None], sh)
    wy = np.broadcast_to(wys[:, :, :, :, None, None], sh).astype(np.float32)
    xl = np.broadcast_to(meta['xl'][:, None, None, None, :, :], sh)
    hx = np.broadcast_to(meta['hx'][:, None, None, None, :, :], sh).astype(np.float32)
    lx = np.broadcast_to(meta['lx'][:, None, None, None, :, :], sh).astype(np.float32)
    mm = np.broadcast_to(m[:, None], sh).astype(np.float32)
    flat = (row * W + xl).reshape(N, REAL)
    w = np.zeros((N, REAL, 2), np.float32)
    w[:, :, 0] = (wy * hx * mm * np.float32(0.25)).reshape(N, REAL)
    w[:, :, 1] = (wy * lx * mm * np.float32(0.25)).reshape(N, REAL)
    return flat, w


def _build_reg(meta, lv):
    """L2/L3: bounding-region pixels + separable host-baked weights.
    Returns idx [N, NJ] int64 (pad idx=0), lhsT [N, NJ, 49] fp32."""
    N = meta['yl'].shape[0]
    H, W, WREG = lv['H'], lv['W'], lv['WREG']
    NJ, RL = lv['NJ'], lv['REAL']
    f = np.float32
    y_base = np.floor(np.clip(meta['y'].reshape(N, -1).min(1), 0.0, H - 1)).astype(np.int64)
    x_base = np.floor(np.clip(meta['x'].reshape(N, -1).min(1), 0.0, W - 1)).astype(np.int64)
    WY = np.zeros((N, WREG, POOLED), np.float32)
    WX = np.zeros((N, WREG, POOLED), np.float32)
    ridx = np.arange(N)[:, None, None]
    pidx = np.broadcast_to(np.arange(POOLED)[None, :, None], (N, POOLED, SAMP))
    np.add.at(WY, (ridx, meta['yl'] - y_base[:, None, None], pidx),
              (f(0.5) * meta['hy'] * meta['masky']).astype(np.float32))
    np.add.at(WY, (ridx, meta['yh'] - y_base[:, None, None], pidx),
              (f(0.5) * meta['ly'] * meta['masky']).astype(np.float32))
    np.add.at(WX, (ridx, meta['xl'] - x_base[:, None, None], pidx),
              (f(0.5) * meta['hx'] * meta['maskx']).astype(np.float32))
    np.add.at(WX, (ridx, meta['xh'] - x_base[:, None, None], pidx),
              (f(0.5) * meta['lx'] * meta['maskx']).astype(np.float32))
    lhsT = np.einsum('nap,nbq->nabpq', WY, WX).reshape(N, RL, NBIN)
    dy = np.arange(WREG)
    idx = ((y_base[:, None, None] + dy[None, :, None]) * W
           + x_base[:, None, None] + dy[None, None, :]).reshape(N, RL)
    idx_full = np.zeros((N, NJ), np.int64)
    lhsT_full = np.zeros((N, NJ, NBIN), np.float32)
    idx_full[:, :RL] = idx
    lhsT_full[:, :RL] = lhsT
    return idx_full, lhsT_full


def _pack_idx(jlists):
    """Pack idx list [..., NJ_total] -> [..., 128, NJ_total//16] int16
    wrapped in 16 partitions, replicated 8x."""
    jl = np.asarray(jlists)
    n = jl.shape[-1]
    arr = jl.reshape(*jl.shape[:-1], n // 16, 16)   # [..., col, p]
    arr = np.swapaxes(arr, -1, -2)                  # [..., p(16), col]
    arr = np.broadcast_to(arr[..., None, :, :],
                          (*jl.shape[:-1], 8, 16, n // 16))
    return arr.reshape(*jl.shape[:-1], 128, n // 16).astype(np.int16)


def _patterns():
    """PATP [128, 2, 49]: L0 parity-sample pattern (k=(py,sy,px,sx) order,
    196 real). PATM [128, 3, 49]: L1 main (j 0..383 of the 392-order).
    PATT [128, 16, 49]: L1 tail variants."""
    k = np.arange(2 * 128)
    px = (k // 2) % 7
    py = k // 28
    patp = np.zeros((2 * 128, NBIN), np.float32)
    v = k < NSAMP
    patp[np.arange(2 * 128)[v], (py * 7 + px)[v]] = 1.0
    patp = patp.reshape(2, 128, NBIN).transpose(1, 0, 2)

    j = np.arange(MAIN)
    px = (j // 2) % 7
    py = (j // 28) % 7
    patm = np.zeros((MAIN, NBIN), np.float32)
    patm[np.arange(MAIN), py * 7 + px] = 1.0
    patm = patm.reshape(3, 128, NBIN).transpose(1, 0, 2)

    patt = np.zeros((128, 16, NBIN), np.float32)
    for r in range(16):
        for kk in range(TAIL):
            patt[r * TAIL + kk, r, TAIL_BINS[kk]] = 1.0
    return patp, patm, patt


def _host_prepare(x0, x1, x2, x3, boxes):
    """Build all per-core input tensors. Returns list of 8 dicts."""
    B = boxes.shape[0]
    # f0 split by row parity
    f0b = np.transpose(np.asarray(x0, np.float32), (0, 2, 3, 1))   # [B,H,W,C]
    f0e = np.zeros((B, F0P_ROWS, C), BF16)
    f0o = np.zeros((B, F0P_ROWS, C), BF16)
    f0e[:, :100 * 200] = f0b[:, 0::2].reshape(B, -1, C).astype(BF16)
    f0o[:, :100 * 200] = f0b[:, 1::2].reshape(B, -1, C).astype(BF16)
    feats = [f0e, f0o]
    dt23 = F8 if FP8_L23 else BF16
    for arr, lv, rows, dt in ((x1, L1, F1_ROWS, BF16), (x2, L2, F2_ROWS, dt23),
                              (x3, L3, F3_ROWS, dt23)):
        f = np.zeros((B, rows, C), dt)
        hw = lv['H'] * lv['W']
        f[:, :hw] = np.ascontiguousarray(
            np.transpose(np.asarray(arr, np.float32), (0, 2, 3, 1))
        ).reshape(B, hw, C).astype(dt)
        feats.append(f)

    per_batch = []
    for b in range(B):
        bb = np.asarray(boxes[b], np.float32)
        m0 = _sample_meta(bb, L0['H'], L0['W'], L0['scale'])
        m1 = _sample_meta(bb, L1['H'], L1['W'], L1['scale'])
        m2 = _sample_meta(bb, L2['H'], L2['W'], L2['scale'])
        m3 = _sample_meta(bb, L3['H'], L3['W'], L3['scale'])
        i0e, w0e, i0o, w0o = _build_l0_parity(m0, L0)
        idx1, w1 = _build_px2(m1, L1)
        idx2, lt2 = _build_reg(m2, L2)
        idx3, lt3 = _build_reg(m3, L3)
        per_batch.append((i0e, w0e, i0o, w0o, idx1, w1, idx2, lt2, idx3, lt3))

    patp, patm, patt = _patterns()

    in_maps = []
    for k in range(8):
        b = k // 4
        s = (k % 4) * NROI_CORE
        i0e, w0e, i0o, w0o, idx1, w1, idx2, lt2, idx3, lt3 = per_batch[b]
        sl = slice(s, s + NROI_CORE)

        cst = np.zeros((128, CST_COLS), BF16)
        patwm = np.concatenate(
            [patp[:, [(k_ % 4) // 2 for k_ in range(8)], :],
             patm[:, [k_ // 2 for k_ in range(6)], :]], axis=1)  # [128,14,49]
        cst[:, PATWM_OFF:PATWM_OFF + 14 * NBIN] = patwm.reshape(128, -1).astype(BF16)
        patwt = np.broadcast_to(patt[:, :, None, :], (128, 16, 2, NBIN))
        cst[:, PATWT_OFF:PATWT_OFF + 16 * 2 * NBIN] = patwt.reshape(128, -1).astype(BF16)

        # wcol [128, roi*16 + k]
        wcol = np.zeros((128, WCOL_COLS), np.float32)
        cols = np.arange(NROI_CORE) * WPR
        # L0: pad [N,196,2] -> [N,256,2]; col par*4 + c*2 + s
        for par, warr in ((0, w0e[sl]), (1, w0o[sl])):
            wp_ = np.zeros((NROI_CORE, L0P, 2), np.float32)
            wp_[:, :NSAMP] = warr
            wp_ = wp_.reshape(NROI_CORE, 2, 128, 2)          # [roi, c, p, s]
            for c in range(2):
                for s2 in range(2):
                    wcol[:, cols + par * 4 + c * 2 + s2] = wp_[:, c, :, s2].T
        # L1 main
        w1c = w1[sl]
        wm1 = w1c[:, :MAIN].reshape(NROI_CORE, 3, 128, 2)
        for c in range(3):
            for s2 in range(2):
                wcol[:, cols + 8 + c * 2 + s2] = wm1[:, c, :, s2].T
        # L1 tail: value at partition (roi%16)*8 + kk
        prt = (np.arange(NROI_CORE) % 16)[:, None] * TAIL + np.arange(TAIL)[None, :]
        for s2 in range(2):
            wt_ = np.zeros((NROI_CORE, 128), np.float32)
            np.put_along_axis(wt_, prt, w1c[:, MAIN:, s2], axis=1)
            wcol[:, cols + 14 + s2] = wt_.T
        wcol = wcol.astype(BF16)

        # idx lists
        def padl(a, n):
            out = np.zeros((a.shape[0], n), np.int64)
            out[:, :a.shape[1]] = a
            return out

        idxs = np.zeros((128, IDX_COLS), np.int16)
        idxs[:, IDX0E_OFF:IDX0E_OFF + NBLK * IC0] = _pack_idx(
            padl(i0e[sl], L0P).reshape(NBLK, BLK * L0P)
        ).transpose(1, 0, 2).reshape(128, -1)
        idxs[:, IDX0O_OFF:IDX0O_OFF + NBLK * IC0] = _pack_idx(
            padl(i0o[sl], L0P).reshape(NBLK, BLK * L0P)
        ).transpose(1, 0, 2).reshape(128, -1)
        idxs[:, IDX1_OFF:IDX1_OFF + NBLK * IC1] = _pack_idx(
            idx1[sl][:, :MAIN].reshape(NBLK, BLK * MAIN)
        ).transpose(1, 0, 2).reshape(128, -1)
        idxs[:, IDX2_OFF:IDX2_OFF + NBLK * IC2] = _pack_idx(
            idx2[sl].reshape(NBLK, BLK * L2['NJ'])
        ).transpose(1, 0, 2).reshape(128, -1)
        idxs[:, IDX3_OFF:IDX3_OFF + NBLK * IC3] = _pack_idx(
            idx3[sl].reshape(NBLK, BLK * L3['NJ'])
        ).transpose(1, 0, 2).reshape(128, -1)
        idxs[:, IDXT1_OFF:IDXT1_OFF + ICT] = _pack_idx(
            idx1[sl][:, MAIN:].reshape(NROI_CORE * TAIL))

        # lhsT k-major: lt2 [roi, 3*128, 49] -> [roi, 128, 3, 49];
        # lt3 [roi, 128, 49]; combined [roi, 128, 4, 49] bf16
        lt2k = lt2[sl].reshape(NROI_CORE, L2['NCH'], 128, NBIN).transpose(0, 2, 1, 3)
        lt3k = lt3[sl].reshape(NROI_CORE, 1, 128, NBIN).transpose(0, 2, 1, 3)
        lt = np.ascontiguousarray(
            np.concatenate([lt2k, lt3k], axis=2)).astype(dt23)

        in_maps.append({
            "f0e": feats[0][b], "f0o": feats[1][b], "f1": feats[2][b],
            "f2": feats[3][b], "f3": feats[4][b],
            "cst": cst, "wcol": wcol, "idxs": idxs, "lt": lt,
        })
    return in_maps


def _build_module():
    from concourse import bacc, tile
    from concourse.bass import mybir
    import concourse.bass as bass_mod

    F32 = mybir.dt.float32
    BF = mybir.dt.bfloat16
    D23 = mybir.dt.float8e4 if FP8_L23 else BF
    I16 = mybir.dt.int16
    AP = bass_mod.AP

    nc = bacc.Bacc(None, target_bir_lowering=False, num_swdge_queues=4)
    f0e = nc.dram_tensor("f0e", [F0P_ROWS, C], BF, kind="ExternalInput")
    f0o = nc.dram_tensor("f0o", [F0P_ROWS, C], BF, kind="ExternalInput")
    f1 = nc.dram_tensor("f1", [F1_ROWS, C], BF, kind="ExternalInput")
    f2 = nc.dram_tensor("f2", [F2_ROWS, C], D23, kind="ExternalInput")
    f3 = nc.dram_tensor("f3", [F3_ROWS, C], D23, kind="ExternalInput")
    cst = nc.dram_tensor("cst", [128, CST_COLS], BF, kind="ExternalInput")
    wcol = nc.dram_tensor("wcol", [128, WCOL_COLS], BF, kind="ExternalInput")
    idxs = nc.dram_tensor("idxs", [128, IDX_COLS], I16, kind="ExternalInput")
    lt = nc.dram_tensor("lt", [NROI_CORE, 128, 4, NBIN], D23, kind="ExternalInput")
    out = nc.dram_tensor("out", [NROI_CORE, NBIN, C], BF, kind="ExternalOutput")

    # 2-px-elem views at 1-px stride
    f0eV = AP(f0e, 0, [[C, F0P_ROWS - 2], [1, 2 * C]])
    f0oV = AP(f0o, 0, [[C, F0P_ROWS - 2], [1, 2 * C]])
    f1v = AP(f1, 0, [[C, F1_ROWS - 2], [1, 2 * C]])

    with tile.TileContext(nc) as tc:
        with (
            tc.tile_pool(name="const", bufs=1) as constp,
            tc.tile_pool(name="g0ep", bufs=2) as g0ep,
            tc.tile_pool(name="g0op", bufs=2) as g0op,
            tc.tile_pool(name="g1p", bufs=2) as g1p,
            tc.tile_pool(name="g2p", bufs=2) as g2p,
            tc.tile_pool(name="g3p", bufs=2) as g3p,
            tc.tile_pool(name="ltp", bufs=3) as ltp,
            tc.tile_pool(name="wp", bufs=4) as wp,
            tc.tile_pool(name="accp", bufs=4, space="PSUM") as accp,
            tc.tile_pool(name="evp", bufs=3) as evp,
        ):
            cst_t = constp.tile([128, CST_COLS], BF)
            nc.sync.dma_start(cst_t[:], cst[:])
            wcol_t = constp.tile([128, WCOL_COLS], BF, tag="wcol")
            nc.sync.dma_start(wcol_t[:], wcol[:])
            idx_t = constp.tile([128, IDX_COLS], I16)
            nc.sync.dma_start(idx_t[:], idxs[:])

            # upfront L1 tail gather: all 128 ROIs' last-8 j's
            qn = [0]

            def nextq():
                q = qn[0] % 4
                qn[0] += 1
                return q

            g1T = constp.tile([128, NROI_CORE // 16, 2 * C], BF, tag="g1T")
            nc.gpsimd.dma_gather(
                g1T[:], f1v, idx_t[:, IDXT1_OFF:IDXT1_OFF + ICT],
                NROI_CORE * TAIL, NROI_CORE * TAIL, 2 * C, elem_step=C,
                queue_num=nextq())

            for blk in range(NBLK):
                # L0 parity gathers: BLK*256 = 1024 descs each (at the cap)
                gt0e = g0ep.tile([128, BLK * 2, 2 * C], BF, tag="g0e")
                io = IDX0E_OFF + blk * IC0
                nc.gpsimd.dma_gather(
                    gt0e[:], f0eV, idx_t[:, io:io + IC0],
                    BLK * L0P, BLK * L0P, 2 * C, elem_step=C,
                    queue_num=nextq())
                gt0o = g0op.tile([128, BLK * 2, 2 * C], BF, tag="g0o")
                io = IDX0O_OFF + blk * IC0
                nc.gpsimd.dma_gather(
                    gt0o[:], f0oV, idx_t[:, io:io + IC0],
                    BLK * L0P, BLK * L0P, 2 * C, elem_step=C,
                    queue_num=nextq())
                # L1/L2: 2-ROI halves keep calls at 768 descs
                gt1 = g1p.tile([128, BLK * 3, 2 * C], BF, tag="g1")
                io = IDX1_OFF + blk * IC1
                for h in range(2):
                    nc.gpsimd.dma_gather(
                        gt1[:, h * 6:(h + 1) * 6, :], f1v,
                        idx_t[:, io + h * 48:io + (h + 1) * 48],
                        2 * MAIN, 2 * MAIN, 2 * C, elem_step=C,
                        queue_num=nextq())
                gt2 = g2p.tile([128, BLK * 3, C], D23, tag="g2")
                io = IDX2_OFF + blk * IC2
                for h in range(2):
                    nc.gpsimd.dma_gather(
                        gt2[:, h * 6:(h + 1) * 6, :], f2[:],
                        idx_t[:, io + h * 48:io + (h + 1) * 48],
                        2 * L2['NJ'], 2 * L2['NJ'], C, elem_step=C,
                        queue_num=nextq())
                gt3 = g3p.tile([128, BLK, C], D23, tag="g3")
                io = IDX3_OFF + blk * IC3
                nc.gpsimd.dma_gather(
                    gt3[:], f3[:], idx_t[:, io:io + IC3],
                    BLK * L3['NJ'], BLK * L3['NJ'], C, elem_step=C,
                    queue_num=nextq())

                for bri in range(BLK):
                    roi = blk * BLK + bri
                    tr = roi % 16            # tail pattern index
                    tch = roi // 16          # tail chunk
                    lt_t = ltp.tile([128, 4, NBIN], D23, tag="lt")
                    nc.sync.dma_start(lt_t[:], lt[roi])

                    acc = accp.tile([NBIN, C], F32)
                    n_mm = 8 + 8 + 3 + 1
                    mi = 0
                    wb = roi * WPR

                    # build all weight mats in 2 DVE ops: pattern blocks *
                    # wcol columns broadcast (0-stride) across the 49 bins
                    wm = wp.tile([128, 14, NBIN], BF, tag="wm")
                    wsl = wcol_t[:, wb:wb + 14]
                    nc.vector.tensor_mul(
                        wm[:],
                        cst_t[:, PATWM_OFF:PATWM_OFF + 14 * NBIN].rearrange(
                            "p (k b) -> p k b", b=NBIN),
                        AP(wsl.tensor, wsl.offset, [*wsl.ap, [0, NBIN]]))
                    wt = wp.tile([128, 2, NBIN], BF, tag="wt")
                    tsl = wcol_t[:, wb + 14:wb + 16]
                    to = PATWT_OFF + tr * 2 * NBIN
                    nc.vector.tensor_mul(
                        wt[:],
                        cst_t[:, to:to + 2 * NBIN].rearrange(
                            "p (k b) -> p k b", b=NBIN),
                        AP(tsl.tensor, tsl.offset, [*tsl.ap, [0, NBIN]]))

                    # L0: 2 parities x 2 chunks x 2 slots
                    for par, gt0 in ((0, gt0e), (1, gt0o)):
                        for c in range(2):
                            for s2 in range(2):
                                nc.tensor.matmul(
                                    acc[:], wm[:, par * 4 + c * 2 + s2, :],
                                    gt0[:, bri * 2 + c, s2 * C:(s2 + 1) * C],
                                    start=(mi == 0), stop=(mi == n_mm - 1))
                                mi += 1
                    # L1 main: 3 chunks x 2 slots
                    for c in range(3):
                        for s2 in range(2):
                            nc.tensor.matmul(
                                acc[:], wm[:, 8 + c * 2 + s2, :],
                                gt1[:, bri * 3 + c, s2 * C:(s2 + 1) * C],
                                start=(mi == 0), stop=(mi == n_mm - 1))
                            mi += 1
                    # L1 tail: 2 slots
                    for s2 in range(2):
                        nc.tensor.matmul(
                            acc[:], wt[:, s2, :], g1T[:, tch, s2 * C:(s2 + 1) * C],
                            start=(mi == 0), stop=(mi == n_mm - 1))
                        mi += 1
                    # L2: 3 chunks, host-baked lhsT
                    for c in range(3):
                        nc.tensor.matmul(
                            acc[:], lt_t[:, c, :], gt2[:, bri * 3 + c, :],
                            start=(mi == 0), stop=(mi == n_mm - 1))
                        mi += 1
                    # L3: 1 chunk
                    nc.tensor.matmul(
                        acc[:], lt_t[:, 3, :], gt3[:, bri, :],
                        start=(mi == 0), stop=(mi == n_mm - 1))
                    mi += 1

                    ev = evp.tile([NBIN, C], BF, tag="ev")
                    nc.scalar.copy(ev[:], acc[:])
                    nc.sync.dma_start(out[roi], ev[:])
    nc.finalize()
    return nc


def bench(iters=12):
    """Device-time estimate: repeat-run the compiled module with pre-staged
    device inputs (no donation). Returns (per_iter_times_s, chained_avg_s)."""
    import time
    import jax
    from jax.experimental.shard_map import shard_map
    from jax.sharding import Mesh, NamedSharding, PartitionSpec
    from concourse import bass2jax
    from concourse.bass import mybir

    nc = _MODULE_CACHE['nc']
    in_maps = LAST_RESULT['in_maps']
    bass2jax.install_neuronx_cc_hook()
    pname = nc.partition_id_tensor.name if nc.partition_id_tensor else None
    in_names, out_names, out_avals, zero_outs = [], [], [], []
    for alloc in nc.m.functions[0].allocations:
        if not isinstance(alloc, mybir.MemoryLocationSet):
            continue
        name = alloc.memorylocations[0].name
        if alloc.kind == "ExternalInput":
            if name != pname:
                in_names.append(name)
        elif alloc.kind == "ExternalOutput":
            shape = tuple(alloc.tensor_shape)
            dtype = mybir.dt.np(alloc.dtype)
            out_names.append(name)
            out_avals.append(jax.core.ShapedArray(shape, dtype))
            zero_outs.append(np.zeros(shape, dtype))
    n_params = len(in_names)
    in_names_all = in_names + out_names + ([pname] if pname else [])

    def _body(*args):
        operands = list(args)
        if pname is not None:
            operands.append(bass2jax.partition_id_tensor())
        return tuple(bass2jax._bass_exec_p.bind(
            *operands,
            out_avals=tuple(out_avals),
            in_names=tuple(in_names_all),
            out_names=tuple(out_names),
            lowering_input_output_aliases=(),
            sim_require_finite=True,
            sim_require_nnan=True,
            nc=nc,
        ))

    n_cores = 8
    devices = jax.devices()[:n_cores]
    mesh = Mesh(np.asarray(devices), ("core",))
    nio = n_params + len(out_names)
    fn = jax.jit(
        shard_map(_body, mesh=mesh, in_specs=(PartitionSpec("core"),) * nio,
                  out_specs=(PartitionSpec("core"),) * len(out_names),
                  check_rep=False),
        keep_unused=True)
    per_core = [[np.asarray(m[name]) for name in in_names] for m in in_maps]
    concat_in = [np.concatenate([per_core[c][i] for c in range(n_cores)], axis=0)
                 for i in range(n_params)]
    concat_zeros = [np.zeros((n_cores * z.shape[0], *z.shape[1:]), z.dtype)
                    for z in zero_outs]
    shard = NamedSharding(mesh, PartitionSpec("core"))
    dev_in = [jax.device_put(a, shard) for a in concat_in + concat_zeros]
    outs = fn(*dev_in)
    jax.block_until_ready(outs)
    times = []
    for _ in range(iters):
        t0 = time.perf_counter()
        outs = fn(*dev_in)
        jax.block_until_ready(outs)
        times.append(time.perf_counter() - t0)
    t0 = time.perf_counter()
    outs_list = [fn(*dev_in) for _ in range(iters)]
    jax.block_until_ready(outs_list)
    chained = (time.perf_counter() - t0) / iters
    return times, chained


def bench_floor(iters=12):
    """Chained-dispatch floor: same bench loop on a trivial copy kernel.
    Subtracting this from bench()'s chained average isolates device time."""
    import time
    import jax
    from jax.experimental.shard_map import shard_map
    from jax.sharding import Mesh, NamedSharding, PartitionSpec
    from concourse import bacc, tile, bass2jax
    from concourse.bass import mybir

    if 'tiny' not in _MODULE_CACHE:
        F32 = mybir.dt.float32
        nct = bacc.Bacc(None, target_bir_lowering=False)
        xin = nct.dram_tensor("xin", [128, 128], F32, kind="ExternalInput")
        xout = nct.dram_tensor("xout", [128, 128], F32, kind="ExternalOutput")
        with tile.TileContext(nct) as tc:
            with tc.tile_pool(name="p", bufs=1) as p:
                t = p.tile([128, 128], F32)
                nct.sync.dma_start(t[:], xin[:])
                nct.sync.dma_start(xout[:], t[:])
        nct.finalize()
        _MODULE_CACHE['tiny'] = nct
    nct = _MODULE_CACHE['tiny']
    bass2jax.install_neuronx_cc_hook()
    pname = nct.partition_id_tensor.name if nct.partition_id_tensor else None
    out_avals = [jax.core.ShapedArray((128, 128), np.float32)]

    def _body(*args):
        operands = list(args)
        if pname is not None:
            operands.append(bass2jax.partition_id_tensor())
        return tuple(bass2jax._bass_exec_p.bind(
            *operands,
            out_avals=tuple(out_avals),
            in_names=("xin", "xout") + ((pname,) if pname else ()),
            out_names=("xout",),
            lowering_input_output_aliases=(),
            sim_require_finite=True,
            sim_require_nnan=True,
            nc=nct,
        ))

    n_cores = 8
    devices = jax.devices()[:n_cores]
    mesh = Mesh(np.asarray(devices), ("core",))
    fn = jax.jit(
        shard_map(_body, mesh=mesh, in_specs=(PartitionSpec("core"),) * 2,
                  out_specs=(PartitionSpec("core"),), check_rep=False),
        keep_unused=True)
    shard = NamedSharding(mesh, PartitionSpec("core"))
    dev_in = [jax.device_put(np.zeros((n_cores * 128, 128), np.float32), shard)
              for _ in range(2)]
    outs = fn(*dev_in)
    jax.block_until_ready(outs)
    t0 = time.perf_counter()
    outs_list = [fn(*dev_in) for _ in range(iters)]
    jax.block_until_ready(outs_list)
    return (time.perf_counter() - t0) / iters


def kernel(x0, x1, x2, x3, boxes):
    from concourse.bass_utils import run_bass_kernel_spmd
    in_maps = _host_prepare(x0, x1, x2, x3, boxes)
    if 'nc' not in _MODULE_CACHE:
        _MODULE_CACHE['nc'] = _build_module()
    nc = _MODULE_CACHE['nc']
    res = run_bass_kernel_spmd(nc, in_maps, list(range(8)), trace=TRACE)
    LAST_RESULT['res'] = res
    LAST_RESULT['in_maps'] = in_maps
    outs = [res.results[k]["out"] for k in range(8)]
    full = np.concatenate(outs, axis=0).astype(np.float32)   # [1024, 49, 256]
    return np.ascontiguousarray(
        full.transpose(0, 2, 1)).reshape(1024, C, POOLED, POOLED)
